# revision 38
# baseline (speedup 1.0000x reference)
"""Tensor-parallel MiniGPT single-token decode step on 8 Trainium2 NeuronCores.

Sharding (per core i of 8):
  - attention: heads 2i, 2i+1 (head_dim 128 -> cols i*256:(i+1)*256 of E=2048);
    wq/wk/wv row-sharded, wo column-sharded, KV cache column-sharded by head.
  - MLP: w1 row-sharded (1024 rows/core), w2 column-sharded.
  - LM head: vocab-sharded (50257 padded to 8*6283=50264 rows).
  - Two 8KB AllGathers + on-core rank reduction combine the wo- and w2-
    partial sums; logits are gathered on the host.

Memory-bound regime: all streamed weights are narrow on the wire.
  - fp8 e4m3: wqkv, K cache, V cache, wo  (attention output is ~1% of the
    residual stream, so 3.6% quantization RMS there is ~4e-4 end to end).
    wqkv/wo are pre-scaled x1024 on the host so sigma=0.02 weights land in
    e4m3's normal range; the 1/1024 is folded into the bf16 activation.
  - bf16: w1, w2, lm_head (these feed logits directly; bf16 keeps ~1e-3).

Engine plan: every matvec contraction runs on the PE (bf16/fp8 moving
operand streams 128 elements/cycle); attention scores are computed
directly in transposed [t-in-block, block] form by loading each K block
as the PE stationary operand, which kills the row->column SBUF-DMA
transpose pipeline. PV accumulates per head with N=128 matmuls
(lhsT = one exp column per t-block). The lm_head shard is k-innermost 512-column
blocks (one contiguous 2MB DMA = one PSUM chain); four of the 13 blocks
run on the DVE (scalar_tensor_tensor accumulate + a deferred ones-matmul
partition reduce) so the post-AllReduce tail is not PE-serial. One shared
9-slot SBUF stream ring lets the lm stream run ahead ~10MB while the
AllGathers are in flight, keeping the DMA queues busy end to end.
"""

import numpy as np

N_CORES = 8
E = 2048
HPC = 2  # heads per core
EPC = HPC * 128  # 256
T = 8192
VOCAB = 50257
VPC = 6283  # padded vocab rows per core (8 * 6283 = 50264)
SCALE = float(1.0 / np.sqrt(128.0))
EPS = 1e-5
WS = 1024.0  # fp8 pre-scale for wqkv / wo

# lm_head chain blocks: (col offset within shard, width)
LM_BLOCKS = [(i * 512, 512) for i in range(12)] + [(6144, VPC - 6144)]
DVE_BLOCKS = (0, 2)  # lm blocks accumulated on the DVE instead of PE

_CACHE = {}
TRACE = False


def _build_nc():
    import concourse.bacc as bacc
    import concourse.mybir as mybir
    import concourse.tile as tile
    from concourse.masks import make_identity

    AF = mybir.ActivationFunctionType
    MUL = mybir.AluOpType.mult
    ADD = mybir.AluOpType.add
    dt = mybir.dt.float32
    bf = mybir.dt.bfloat16
    f8 = mybir.dt.float8e4

    nc = bacc.Bacc(
        "TRN2", target_bir_lowering=False, debug=False, num_devices=N_CORES
    )

    xe_wte = nc.declare_dram_parameter("xe_wte", [128, 16], dt, isOutput=False)
    xe_wpe = nc.declare_dram_parameter("xe_wpe", [128, 16], dt, isOutput=False)
    wqkv_r = nc.declare_dram_parameter("wqkv_r", [128, 16 * 768], f8, isOutput=False)
    keys_r = nc.declare_dram_parameter("keys_r", [128, 2 * 8192], f8, isOutput=False)
    vals_r = nc.declare_dram_parameter("vals_r", [128, 64 * 256], f8, isOutput=False)
    wo_r = nc.declare_dram_parameter("wo_r", [128, 2 * 2048], f8, isOutput=False)
    w1_r = nc.declare_dram_parameter("w1_r", [128, 16 * 1024], bf, isOutput=False)
    w2_r = nc.declare_dram_parameter("w2_r", [128, 8 * 2048], bf, isOutput=False)
    lm_r = nc.declare_dram_parameter("lm_r", [128, 16 * VPC], bf, isOutput=False)
    logits_out = nc.declare_dram_parameter("logits", [1, VPC], dt, isOutput=True)

    with tile.TileContext(nc) as tc:
        with (
            tc.tile_pool(name="const", bufs=1) as const,
            tc.tile_pool(name="small", bufs=1) as small,
            tc.tile_pool(name="stage", bufs=2) as stage,
            tc.tile_pool(name="ps", bufs=7, space="PSUM") as ps,
            tc.tile_pool(name="dram", bufs=1, space="DRAM") as dram,
            tc.tile_pool(name="stream", bufs=9) as stream,
            tc.tile_pool(name="acc", bufs=2) as accp,
        ):
            _snum = [0]

            def stile(label, width, dtype):
                # one shared ring of slots; slot size = max tile = 16KB/part
                _snum[0] += 1
                return stream.tile(
                    [128, width], dtype, tag="s", name=f"s{_snum[0]}_{label}"
                )

            # Warm up the collectives path first: a NEFF's first collective
            # pays a large ncfw init (~60-80us observed) that would otherwise
            # land on AR1's critical path. Its output readback + DCE-keeper
            # are emitted at the very END of the program so no engine FIFO
            # ever blocks on this collective's completion.
            warm_in = dram.tile([1, 16], dt, tag="warm_in")
            warm_out = dram.tile([N_CORES, 16], dt, tag="warm_out")
            warm_sb = stage.tile([1, 16], dt, tag="warm", bufs=1)
            nc.vector.memset(warm_sb[:], 0.0)
            nc.gpsimd.dma_start(warm_in[:], warm_sb[:])
            nc.gpsimd.collective_compute(
                "AllGather",
                mybir.AluOpType.bypass,
                replica_groups=[list(range(N_CORES))],
                ins=[warm_in.opt()],
                outs=[warm_out.opt()],
            )

            # ---- embedding row loads lead the weight-stream queue ----
            xw = stage.tile([128, 16], dt, tag="xw")
            nc.sync.dma_start(xw[:], xe_wte[:])
            xp = stage.tile([128, 16], dt, tag="xp")
            nc.sync.dma_start(xp[:], xe_wpe[:])

            ones_col = const.tile([128, 1], dt)
            nc.vector.memset(ones_col[:], 1.0)
            ones_row = const.tile([1, 128], dt)
            nc.vector.memset(ones_row[:], 1.0)
            ones_row_ws = const.tile([1, 128], dt)
            nc.vector.memset(ones_row_ws[:], 1.0 / WS)
            ident = const.tile([16, 16], dt)
            make_identity(nc, ident[:])
            eps_c = const.tile([1, 1], dt)
            nc.vector.memset(eps_c[:], EPS)
            ones_bf = const.tile([128, 1], bf)
            nc.vector.memset(ones_bf[:], 1.0)

            wscr = const.tile([128, 512], bf)
            nc.vector.memset(wscr[:], 0.25)

            def warm(n):
                # keep-the-HAM-warm dummies: cheap bf16 matmuls on resident data
                for _ in range(n):
                    wm = ps.tile([1, 512], dt, tag="wm", bufs=1, name="wm")
                    nc.tensor.matmul(
                        wm[:], ones_bf[:], wscr[:], start=True, stop=True
                    )

            def rms(xt, name, out_dtype=dt, rowc=None):
                """x * rsqrt(mean(x^2) + eps) for x in [128, 16] column layout.

                rowc: [1, 128] broadcast row; its value multiplies the
                rsqrt scale (used to fold the fp8 weight pre-scale in).
                """
                sq = small.tile([128, 16], dt, tag=f"sq_{name}")
                ssum = small.tile([128, 1], dt, tag=f"ss_{name}")
                nc.scalar.activation(sq[:], xt[:], AF.Square, accum_out=ssum[:])
                tot = ps.tile([1, 1], dt, tag="b")
                nc.tensor.matmul(tot[:], ssum[:], ones_col[:], start=True, stop=True)
                std = small.tile([1, 1], dt, tag=f"std_{name}")
                nc.scalar.activation(
                    std[:], tot[:], AF.Sqrt, bias=eps_c[:], scale=1.0 / float(E)
                )
                inv = small.tile([1, 1], dt, tag=f"inv_{name}")
                nc.vector.reciprocal(inv[:], std[:])
                invb_ps = ps.tile([128, 1], dt, tag="b")
                nc.tensor.matmul(
                    invb_ps[:], rowc if rowc is not None else ones_row[:],
                    inv[:], start=True, stop=True,
                )
                xn = small.tile([128, 16], out_dtype, tag=f"xn_{name}")
                nc.vector.tensor_scalar_mul(xn[:], xt[:], invb_ps[:])
                return xn

            x0 = small.tile([128, 16], dt, tag="x0")
            nc.vector.tensor_add(x0[:], xw[:], xp[:])
            x1 = rms(x0, "n1")  # residual input (fp32)
            # second rms emits bf16 with the wqkv fp8 pre-scale folded in
            x2b = rms(x1, "n2", out_dtype=bf, rowc=ones_row_ws[:])

            # ---- qkv projection: [1, 768] row (q 0:256 | k 256:512 | v 512:768)
            wqkv_t = stile("qkv", 16 * 768, f8)
            nc.sync.dma_start(wqkv_t[:], wqkv_r[:])
            ps_q = ps.tile([1, 512], dt, tag="b", name="ps_q")
            ps_v = ps.tile([1, 256], dt, tag="b", name="ps_v")
            for k in range(16):
                wt = wqkv_t[:, k * 768 : (k + 1) * 768]
                nc.tensor.matmul(
                    ps_q[:], x2b[:, k : k + 1], wt[:, 0:512],
                    start=(k == 0), stop=(k == 15),
                )
                nc.tensor.matmul(
                    ps_v[:], x2b[:, k : k + 1], wt[:, 512:768],
                    start=(k == 0), stop=(k == 15),
                )
            qkv_row = small.tile([1, 768], dt, tag="qkv")
            nc.scalar.mul(qkv_row[:, 0:256], ps_q[:, 0:256], SCALE)
            nc.scalar.copy(qkv_row[:, 256:512], ps_q[:, 256:512])
            nc.scalar.copy(qkv_row[:, 512:768], ps_v[:])

            # ---- transpose q,k to column layout: qkT[:, 0:2]=q heads, 2:4=k heads
            st4 = stage.tile([4, 128], dt, tag="st4")
            nc.scalar.dma_start(st4[:], qkv_row[:, 0:512])
            qkT_ps = ps.tile([128, 4], dt, tag="b")
            nc.tensor.transpose(qkT_ps[:], st4[:], ident[0:4, 0:4])
            qkT = small.tile([128, 4], bf, tag="qkT")
            nc.vector.tensor_copy(qkT[:], qkT_ps[:])

            # ---- attention scores, directly transposed: each 128-wide K block
            # is the PE stationary operand, q the (N=1) moving operand, so
            # att lands as [t-in-block, block] columns with no SBUF reshape.
            # wTboth[p, h*64 + c] = exp(att_h[c*128 + p])  (bf16 PV columns)
            wTboth = small.tile([128, 128], bf, tag="wTboth")
            esp2 = small.tile([128, 2], dt, tag="esp2")  # per-partition exp sums
            for h in range(HPC):
                kt = stile("key", 8192, f8)
                nc.sync.dma_start(kt[:], keys_r[:, h * 8192 : (h + 1) * 8192])
                att_ps = ps.tile([128, 64], dt, tag="b", name=f"attps{h}")
                for b in range(64):
                    nc.tensor.matmul(
                        att_ps[:, b : b + 1],
                        kt[:, b * 128 : (b + 1) * 128],
                        qkT[:, h : h + 1],
                        start=True, stop=True,
                    )
                nc.scalar.activation(
                    wTboth[:, h * 64 : (h + 1) * 64], att_ps[:], AF.Exp,
                    accum_out=esp2[:, h : h + 1],
                )

            # current-token score per head: exp(q_h . k_h) (SCALE folded into q)
            e_last = small.tile([1, 2], dt, tag="elast")
            for h in range(HPC):
                pal = ps.tile([1, 1], dt, tag="b")
                nc.tensor.matmul(
                    pal[:], qkT[:, h : h + 1], qkT[:, 2 + h : 3 + h],
                    start=True, stop=True,
                )
                nc.scalar.activation(e_last[:, h : h + 1], pal[:], AF.Exp)

            # softmax denominators: cross-partition sum of esp2 + e_last
            dps = ps.tile([1, 2], dt, tag="b")
            nc.tensor.matmul(dps[:], ones_col[:], esp2[:], start=True, stop=True)
            dtmp = small.tile([1, 2], dt, tag="dtmp")
            nc.vector.tensor_add(dtmp[:], dps[:], e_last[:])
            dinv = small.tile([1, 2], dt, tag="dinv")
            nc.vector.reciprocal(dinv[:], dtmp[:])

            # ---- PV on the PE: lhsT = exp column [128, 1] (t-block on
            # partitions), rhs = V block [t, d]; 64 accumulating N=128
            # matmuls per head.
            pv_ps = [
                ps.tile([1, 128], dt, tag="b", name=f"pv_ps{h}") for h in range(HPC)
            ]
            for tt in range(2):
                vt = stile("val", 8192, f8)
                nc.sync.dma_start(vt[:], vals_r[:, tt * 8192 : (tt + 1) * 8192])
                for j in range(32):
                    c = tt * 32 + j
                    for h in range(HPC):
                        nc.tensor.matmul(
                            pv_ps[h][:],
                            wTboth[:, h * 64 + c : h * 64 + c + 1],
                            vt[:, j * 256 + h * 128 : j * 256 + (h + 1) * 128],
                            start=(c == 0), stop=(c == 63),
                        )

            # combine with current-token value, then normalize by the softmax sum
            xa_row = small.tile([1, 256], dt, tag="xa")
            for h in range(HPC):
                sl = slice(h * 128, (h + 1) * 128)
                nc.vector.tensor_scalar_mul(
                    xa_row[:, sl],
                    qkv_row[:, 512 + h * 128 : 512 + (h + 1) * 128],
                    e_last[:, h : h + 1],
                )
                nc.vector.tensor_add(xa_row[:, sl], xa_row[:, sl], pv_ps[h][:])
                nc.vector.tensor_scalar_mul(xa_row[:, sl], xa_row[:, sl], dinv[:, h : h + 1])

            # ---- transpose x_attn to column layout [128, 2], fold wo pre-scale
            st2 = stage.tile([2, 128], dt, tag="st2")
            nc.scalar.dma_start(st2[:], xa_row[:])
            xaT_ps = ps.tile([128, 2], dt, tag="b")
            nc.tensor.transpose(xaT_ps[:], st2[:], ident[0:2, 0:2])
            xaT = small.tile([128, 2], bf, tag="xaT")
            nc.scalar.mul(xaT[:], xaT_ps[:], 1.0 / WS)

            # ---- wo partial: [1, 2048] = 4 PE chains over 2 k-columns
            ar1_in = small.tile([1, 2048], dt, tag="arin", name="ar1_in")
            wot = stile("wo", 4096, f8)
            nc.sync.dma_start(wot[:], wo_r[:])
            wo_pe = [ps.tile([1, 512], dt, tag="b", name=f"wope{n}") for n in range(4)]
            for k in range(2):
                for n in range(4):
                    nc.tensor.matmul(
                        wo_pe[n][:], xaT[:, k : k + 1],
                        wot[:, k * 2048 + n * 512 : k * 2048 + (n + 1) * 512],
                        start=(k == 0), stop=(k == 1),
                    )
            for n in range(4):
                if n % 2 == 0:
                    nc.vector.tensor_copy(ar1_in[:, n * 512 : (n + 1) * 512], wo_pe[n][:])
                else:
                    nc.scalar.copy(ar1_in[:, n * 512 : (n + 1) * 512], wo_pe[n][:])

            def all_reduce(row_sb, idx, name):
                """Sum a [1, 2048] partial across cores; returns PSUM [128, 16].

                AllGather + on-core rank reduction: 16 K=8 matmuls against a
                ones vector sum the 8 gathered rows and transpose into the
                [128, 16] column layout.
                """
                in_d = dram.tile([1, 2048], dt, tag=f"{name}_in")
                out_d = dram.tile([N_CORES, 2048], dt, tag=f"{name}_out")
                nc.scalar.dma_start(in_d[:], row_sb[:])
                nc.gpsimd.collective_compute(
                    "AllGather",
                    mybir.AluOpType.bypass,
                    replica_groups=[list(range(N_CORES))],
                    ins=[in_d.opt()],
                    outs=[out_d.opt()],
                )
                ag_sb = stage.tile(
                    [N_CORES, 2048], dt, tag="ag", name=f"ag_{name}", bufs=1
                )
                nc.scalar.dma_start(ag_sb[:], out_d[:])
                # DVE delay chain of copies feeding dummy matmuls: spreads
                # PE activity across the collective wait so HAM stays at
                # 8/8 and the rank-reduce below runs warm
                wd_prev = wscr
                for i in range(8):
                    wd = stage.tile(
                        [128, 512], bf, tag="wd", name=f"wd_{name}{i}", bufs=2
                    )
                    nc.vector.tensor_copy(wd[:], wd_prev[:])
                    wm = ps.tile([1, 512], dt, tag="wm", bufs=1, name="wm")
                    nc.tensor.matmul(wm[:], ones_bf[:], wd[:], start=True, stop=True)
                    wd_prev = wd
                x_ps = ps.tile([128, 16], dt, tag="b", name=f"xps_{name}")
                for c in range(16):
                    nc.tensor.matmul(
                        x_ps[:, c : c + 1],
                        ag_sb[:, c * 128 : (c + 1) * 128],
                        ones_col[0:N_CORES, :],
                        start=True, stop=True,
                    )
                return x_ps

            x3_ps = all_reduce(ar1_in, 0, "ar1")
            x3 = small.tile([128, 16], dt, tag="x3")
            nc.vector.tensor_add(x3[:], x3_ps[:], x1[:])  # + residual

            # ---- MLP1: h = relu(w1 @ x4), 2 PE chains over 16 k-columns ----
            x4b = rms(x3, "n3", out_dtype=bf)
            ph = [ps.tile([1, 512], dt, tag="b", name=f"ph{n}") for n in range(2)]
            for a in range(2):
                w1t = stile("w1", 8192, bf)
                nc.sync.dma_start(w1t[:], w1_r[:, a * 8192 : (a + 1) * 8192])
                for b in range(8):
                    k = a * 8 + b
                    for n in range(2):
                        nc.tensor.matmul(
                            ph[n][:], x4b[:, k : k + 1],
                            w1t[:, b * 1024 + n * 512 : b * 1024 + (n + 1) * 512],
                            start=(k == 0), stop=(k == 15),
                        )
            h_row = small.tile([1, 1024], dt, tag="hrow")
            for n in range(2):
                nc.scalar.activation(h_row[:, n * 512 : (n + 1) * 512], ph[n][:], AF.Relu)

            st8 = stage.tile([8, 128], dt, tag="st8")
            nc.scalar.dma_start(st8[:], h_row[:])
            hT_ps = ps.tile([128, 8], dt, tag="b")
            nc.tensor.transpose(hT_ps[:], st8[:], ident[0:8, 0:8])
            hT = small.tile([128, 8], bf, tag="hT")
            nc.vector.tensor_copy(hT[:], hT_ps[:])

            # ---- MLP2: [1, 2048] = 4 PE chains over 8 k-columns ----
            ar2_in = small.tile([1, 2048], dt, tag="arin", name="ar2_in")
            pm = [ps.tile([1, 512], dt, tag="b", name=f"pm{n}") for n in range(4)]
            for a in range(2):
                w2t = stile("w2", 8192, bf)
                nc.sync.dma_start(w2t[:], w2_r[:, a * 8192 : (a + 1) * 8192])
                for b in range(4):
                    k = a * 4 + b
                    for n in range(4):
                        nc.tensor.matmul(
                            pm[n][:], hT[:, k : k + 1],
                            w2t[:, b * 2048 + n * 512 : b * 2048 + (n + 1) * 512],
                            start=(k == 0), stop=(k == 7),
                        )
            for n in range(4):
                if n % 2 == 0:
                    nc.vector.tensor_copy(ar2_in[:, n * 512 : (n + 1) * 512], pm[n][:])
                else:
                    nc.scalar.copy(ar2_in[:, n * 512 : (n + 1) * 512], pm[n][:])

            x5_ps = all_reduce(ar2_in, 1, "ar2")
            x5f = small.tile([128, 16], dt, tag="x5f")
            nc.vector.tensor_add(x5f[:], x5_ps[:], x3[:])  # + residual (x3)
            x5b = small.tile([128, 16], bf, tag="x5b")
            nc.scalar.copy(x5b[:], x5f[:])

            # ---- LM head over the vocab shard: 13 k-innermost column blocks,
            # each one contiguous DMA. PE blocks: one 16-matmul PSUM chain.
            # DVE blocks: 16 scalar_tensor_tensor accumulates into an SBUF acc
            # with the cross-partition ones-matmul reduce deferred so the PE
            # never stalls waiting on the DVE.
            def drain(pl, lo, w, cb):
                lr = small.tile([1, 512], dt, tag="lrow", name=f"lr{cb}", bufs=3)
                nc.vector.tensor_copy(lr[:, 0:w], pl[:])
                nc.scalar.dma_start(logits_out[:, lo : lo + w], lr[:, 0:w])

            pending = []  # deferred DVE-block reduces: (acc, lo, w, cb)
            pe_seen = 0
            off16 = 0
            for cb, (lo, w) in enumerate(LM_BLOCKS):
                lt = stile("lm", 16 * w, bf)
                nc.sync.dma_start(lt[:], lm_r[:, off16 : off16 + 16 * w])
                if cb in DVE_BLOCKS:
                    # bf16 accumulator: both DVE ports 16-bit
                    acc = accp.tile([128, w], bf, tag="acc", name=f"acc{cb}")
                    for k in range(16):
                        if k == 0:
                            nc.vector.tensor_scalar_mul(
                                acc[:], lt[:, 0:w], x5f[:, 0:1]
                            )
                        else:
                            nc.vector.scalar_tensor_tensor(
                                acc[:], lt[:, k * w : (k + 1) * w],
                                x5f[:, k : k + 1], acc[:],
                                op0=MUL, op1=ADD,
                            )
                    pending.append((acc, lo, w, cb))
                else:
                    pe_seen += 1
                    if pe_seen in (4, 8) and pending:
                        acc, plo, pw, pcb = pending.pop(0)
                        pr = ps.tile([1, pw], dt, tag="b", name=f"pr{pcb}")
                        nc.tensor.matmul(
                            pr[:], ones_bf[:], acc[:], start=True, stop=True
                        )
                        drain(pr, plo, pw, pcb)
                    pl = ps.tile([1, w], dt, tag="b", name=f"pl{cb}")
                    for k in range(16):
                        nc.tensor.matmul(
                            pl[:], x5b[:, k : k + 1], lt[:, k * w : (k + 1) * w],
                            start=(k == 0), stop=(k == 15),
                        )
                    drain(pl, lo, w, cb)
                off16 += 16 * w
            assert not pending, "deferred DVE reduces must drain before cb12"

            # warmup-collective readback + DCE-keeper, emitted last so no
            # engine FIFO ever waits on the warmup AG mid-kernel; folds a
            # zero-weighted read of its output into the final logits row.
            warm_back = stage.tile([1, 16], dt, tag="warmb", bufs=1)
            nc.scalar.dma_start(warm_back[:], warm_out[0:1, :])
            klo, kw = LM_BLOCKS[-1]
            kr = small.tile([1, 16], dt, tag="keep")
            nc.vector.scalar_tensor_tensor(
                kr[:], warm_back[:], 0.0, warm_back[:], op0=MUL, op1=MUL
            )
            nc.gpsimd.dma_start(
                logits_out[:, klo + kw - 16 : klo + kw], kr[:],
                accum_op=ADD,
            )

    nc.finalize()
    return nc


def _col16(v):
    """[2048] vector -> [128, 16] column-major layout (e = c*128 + p at [p, c])."""
    return np.ascontiguousarray(v.reshape(16, 128).T)


def _part_major(mT, nblk, blk_rows, width):
    """[nblk*blk_rows, width] -> [blk_rows, nblk*width] partition-major."""
    return np.ascontiguousarray(
        mT.reshape(nblk, blk_rows, width).transpose(1, 0, 2).reshape(blk_rows, nblk * width)
    )


def _to_f8(a):
    import ml_dtypes

    return np.clip(a, -240.0, 240.0).astype(ml_dtypes.float8_e4m3)


def _to_bf(a):
    import ml_dtypes

    return a.astype(ml_dtypes.bfloat16)


def _lm_blocked(shard):
    """[VPC, E] fp32 -> [128, 16*VPC] bf16 with k-innermost 512-col blocks."""
    cols = []
    for lo, w in LM_BLOCKS:
        blk = shard[lo : lo + w].T  # [E, w]
        cols.append(blk.reshape(16, 128, w).transpose(1, 0, 2).reshape(128, 16 * w))
    return _to_bf(np.concatenate(cols, axis=1))


def _prep_in_maps(token_id, pos_id, keys, values, wte, wpe, wq, wk, wv, wo, w1, w2, lm_w):
    f32 = lambda a: np.asarray(a, dtype=np.float32)
    keys, values = f32(keys), f32(values)
    wq, wk, wv, wo, w1, w2, lm_w = map(f32, (wq, wk, wv, wo, w1, w2, lm_w))
    xe_wte = _col16(f32(wte[token_id]))
    xe_wpe = _col16(f32(wpe[pos_id]))
    lm_pad = np.zeros((N_CORES * VPC, E), np.float32)
    lm_pad[:VOCAB] = lm_w

    in_maps = []
    for i in range(N_CORES):
        hs = slice(i * EPC, (i + 1) * EPC)
        wqkv = np.concatenate([wq[hs], wk[hs], wv[hs]], axis=0)  # [768, E]
        in_maps.append(
            {
                "xe_wte": xe_wte,
                "xe_wpe": xe_wpe,
                "wqkv_r": _to_f8(
                    _part_major(np.ascontiguousarray(wqkv.T) * WS, 16, 128, 768)
                ),
                "keys_r": _to_f8(
                    _part_major(np.ascontiguousarray(keys[:, hs].T), 2, 128, 8192)
                ),
                "vals_r": _to_f8(_part_major(values[:, hs], 64, 128, EPC)),
                "wo_r": _to_f8(
                    _part_major(np.ascontiguousarray(wo[:, hs].T) * WS, 2, 128, E)
                ),
                "w1_r": _to_bf(
                    _part_major(
                        np.ascontiguousarray(w1[i * 1024 : (i + 1) * 1024].T),
                        16, 128, 1024,
                    )
                ),
                "w2_r": _to_bf(
                    _part_major(
                        np.ascontiguousarray(w2[:, i * 1024 : (i + 1) * 1024].T),
                        8, 128, E,
                    )
                ),
                "lm_r": _lm_blocked(lm_pad[i * VPC : (i + 1) * VPC]),
            }
        )
    return in_maps


def kernel(**inputs) -> np.ndarray:
    from concourse.bass_utils import run_bass_kernel_spmd

    token_id = int(inputs["token_id"])
    pos_id = int(inputs["pos_id"])
    in_maps = _prep_in_maps(
        token_id,
        pos_id,
        inputs["keys"],
        inputs["values"],
        inputs["wte"],
        inputs["wpe"],
        inputs["wq"],
        inputs["wk"],
        inputs["wv"],
        inputs["wo"],
        inputs["w1"],
        inputs["w2"],
        inputs["lm_w"],
    )
    if "nc" not in _CACHE:
        _CACHE["nc"] = _build_nc()
    nc = _CACHE["nc"]
    res = run_bass_kernel_spmd(
        nc,
        in_maps,
        core_ids=list(range(N_CORES)),
        trace=TRACE,
        trace_cores=[0] if TRACE else None,
    )
    _CACHE["last_result"] = res
    logits = np.concatenate([r["logits"][0] for r in res.results])[:VOCAB]
    return np.ascontiguousarray(logits.astype(np.float32))


# revision 39
# speedup vs baseline: 1.1000x; 1.1000x over previous
"""Tensor-parallel MiniGPT single-token decode step on 8 Trainium2 NeuronCores.

Sharding (per core i of 8):
  - attention: heads 2i, 2i+1 (head_dim 128 -> cols i*256:(i+1)*256 of E=2048);
    wq/wk/wv row-sharded, wo column-sharded, KV cache column-sharded by head.
  - MLP: w1 row-sharded (1024 rows/core), w2 column-sharded.
  - LM head: vocab-sharded (50257 padded to 8*6283=50264 rows).
  - Two 8KB AllGathers + on-core rank reduction combine the wo- and w2-
    partial sums; logits are gathered on the host.

Memory-bound regime: all streamed weights are narrow on the wire.
  - fp8 e4m3: wqkv, K cache, V cache, wo  (attention output is ~1% of the
    residual stream, so 3.6% quantization RMS there is ~4e-4 end to end).
    wqkv/wo are pre-scaled x1024 on the host so sigma=0.02 weights land in
    e4m3's normal range; the 1/1024 is folded into the bf16 activation.
  - bf16: w1, w2, lm_head (these feed logits directly; bf16 keeps ~1e-3).

Engine plan: every matvec contraction runs on the PE (bf16/fp8 moving
operand streams 128 elements/cycle); attention scores are computed
directly in transposed [t-in-block, block] form by loading each K block
as the PE stationary operand, which kills the row->column SBUF-DMA
transpose pipeline. PV accumulates per head with N=128 matmuls
(lhsT = one exp column per t-block). The lm_head shard is k-innermost 512-column
blocks (one contiguous 2MB DMA = one PSUM chain); four of the 13 blocks
run on the DVE (scalar_tensor_tensor accumulate + a deferred ones-matmul
partition reduce) so the post-AllReduce tail is not PE-serial. One shared
9-slot SBUF stream ring lets the lm stream run ahead ~10MB while the
AllGathers are in flight, keeping the DMA queues busy end to end.
"""

import numpy as np

N_CORES = 8
E = 2048
HPC = 2  # heads per core
EPC = HPC * 128  # 256
T = 8192
VOCAB = 50257
VPC = 6283  # padded vocab rows per core (8 * 6283 = 50264)
SCALE = float(1.0 / np.sqrt(128.0))
EPS = 1e-5
WS = 1024.0  # fp8 pre-scale for wqkv / wo

# lm_head chain blocks: (col offset within shard, width)
LM_BLOCKS = [(i * 512, 512) for i in range(12)] + [(6144, VPC - 6144)]
DVE_BLOCKS = (0, 2)  # lm blocks accumulated on the DVE instead of PE

_CACHE = {}
TRACE = False


def _build_nc():
    import concourse.bacc as bacc
    import concourse.mybir as mybir
    import concourse.tile as tile
    from concourse.masks import make_identity

    AF = mybir.ActivationFunctionType
    MUL = mybir.AluOpType.mult
    ADD = mybir.AluOpType.add
    dt = mybir.dt.float32
    bf = mybir.dt.bfloat16
    f8 = mybir.dt.float8e4

    nc = bacc.Bacc(
        "TRN2", target_bir_lowering=False, debug=False, num_devices=N_CORES
    )

    xe_wte = nc.declare_dram_parameter("xe_wte", [128, 16], dt, isOutput=False)
    xe_wpe = nc.declare_dram_parameter("xe_wpe", [128, 16], dt, isOutput=False)
    wqkv_r = nc.declare_dram_parameter("wqkv_r", [128, 16 * 768], f8, isOutput=False)
    keys_r = nc.declare_dram_parameter("keys_r", [128, 2 * 8192], f8, isOutput=False)
    vals_r = nc.declare_dram_parameter("vals_r", [128, 64 * 256], f8, isOutput=False)
    wo_r = nc.declare_dram_parameter("wo_r", [128, 2 * 2048], f8, isOutput=False)
    w1_r = nc.declare_dram_parameter("w1_r", [128, 16 * 1024], bf, isOutput=False)
    w2_r = nc.declare_dram_parameter("w2_r", [128, 8 * 2048], bf, isOutput=False)
    lm_r = nc.declare_dram_parameter("lm_r", [128, 16 * VPC], bf, isOutput=False)
    logits_out = nc.declare_dram_parameter("logits", [1, VPC], dt, isOutput=True)

    with tile.TileContext(nc) as tc:
        with (
            tc.tile_pool(name="const", bufs=1) as const,
            tc.tile_pool(name="small", bufs=1) as small,
            tc.tile_pool(name="stage", bufs=2) as stage,
            tc.tile_pool(name="ps", bufs=7, space="PSUM") as ps,
            tc.tile_pool(name="dram", bufs=1, space="DRAM") as dram,
            tc.tile_pool(name="stream", bufs=9) as stream,
            tc.tile_pool(name="acc", bufs=2) as accp,
        ):
            _snum = [0]

            def stile(label, width, dtype):
                # one shared ring of slots; slot size = max tile = 16KB/part
                _snum[0] += 1
                return stream.tile(
                    [128, width], dtype, tag="s", name=f"s{_snum[0]}_{label}"
                )

            # Warm up the collectives path first: a NEFF's first collective
            # pays a large ncfw init (~60-80us observed) that would otherwise
            # land on AR1's critical path. Its output readback + DCE-keeper
            # are emitted at the very END of the program so no engine FIFO
            # ever blocks on this collective's completion.
            warm_in = dram.tile([1, 16], dt, tag="warm_in")
            warm_out = dram.tile([N_CORES, 16], dt, tag="warm_out")
            warm_sb = stage.tile([1, 16], dt, tag="warm", bufs=1)
            nc.vector.memset(warm_sb[:], 0.0)
            nc.gpsimd.dma_start(warm_in[:], warm_sb[:])
            nc.gpsimd.collective_compute(
                "AllGather",
                mybir.AluOpType.bypass,
                replica_groups=[list(range(N_CORES))],
                ins=[warm_in.opt()],
                outs=[warm_out.opt()],
            )

            # ---- embedding row loads lead the weight-stream queue ----
            xw = stage.tile([128, 16], dt, tag="xw")
            nc.sync.dma_start(xw[:], xe_wte[:])
            xp = stage.tile([128, 16], dt, tag="xp")
            nc.sync.dma_start(xp[:], xe_wpe[:])

            ones_col = const.tile([128, 1], dt)
            nc.vector.memset(ones_col[:], 1.0)
            ones_row = const.tile([1, 128], dt)
            nc.vector.memset(ones_row[:], 1.0)
            ones_row_ws = const.tile([1, 128], dt)
            nc.vector.memset(ones_row_ws[:], 1.0 / WS)
            ident = const.tile([16, 16], dt)
            make_identity(nc, ident[:])
            eps_c = const.tile([1, 1], dt)
            nc.vector.memset(eps_c[:], EPS)
            ones_bf = const.tile([128, 1], bf)
            nc.vector.memset(ones_bf[:], 1.0)

            wscr = const.tile([128, 512], bf)
            nc.vector.memset(wscr[:], 0.25)

            def warm(n):
                # keep-the-HAM-warm dummies: cheap bf16 matmuls on resident data
                for _ in range(n):
                    wm = ps.tile([1, 512], dt, tag="wm", bufs=1, name="wm")
                    nc.tensor.matmul(
                        wm[:], ones_bf[:], wscr[:], start=True, stop=True
                    )

            def rms(xt, name, out_dtype=dt, rowc=None):
                """x * rsqrt(mean(x^2) + eps) for x in [128, 16] column layout.

                rowc: [1, 128] broadcast row; its value multiplies the
                rsqrt scale (used to fold the fp8 weight pre-scale in).
                """
                sq = small.tile([128, 16], dt, tag=f"sq_{name}")
                ssum = small.tile([128, 1], dt, tag=f"ss_{name}")
                nc.scalar.activation(sq[:], xt[:], AF.Square, accum_out=ssum[:])
                tot = ps.tile([1, 1], dt, tag="b")
                nc.tensor.matmul(tot[:], ssum[:], ones_col[:], start=True, stop=True)
                std = small.tile([1, 1], dt, tag=f"std_{name}")
                nc.scalar.activation(
                    std[:], tot[:], AF.Sqrt, bias=eps_c[:], scale=1.0 / float(E)
                )
                inv = small.tile([1, 1], dt, tag=f"inv_{name}")
                nc.vector.reciprocal(inv[:], std[:])
                invb_ps = ps.tile([128, 1], dt, tag="b")
                nc.tensor.matmul(
                    invb_ps[:], rowc if rowc is not None else ones_row[:],
                    inv[:], start=True, stop=True,
                )
                xn = small.tile([128, 16], out_dtype, tag=f"xn_{name}")
                nc.vector.tensor_scalar_mul(xn[:], xt[:], invb_ps[:])
                return xn

            x0 = small.tile([128, 16], dt, tag="x0")
            nc.vector.tensor_add(x0[:], xw[:], xp[:])
            x1 = rms(x0, "n1")  # residual input (fp32)
            # second rms emits bf16 with the wqkv fp8 pre-scale folded in
            x2b = rms(x1, "n2", out_dtype=bf, rowc=ones_row_ws[:])

            # ---- qkv projection: [1, 768] row (q 0:256 | k 256:512 | v 512:768)
            wqkv_t = stile("qkv", 16 * 768, f8)
            nc.sync.dma_start(wqkv_t[:], wqkv_r[:])
            ps_q = ps.tile([1, 512], dt, tag="b", name="ps_q")
            ps_v = ps.tile([1, 256], dt, tag="b", name="ps_v")
            for k in range(16):
                wt = wqkv_t[:, k * 768 : (k + 1) * 768]
                nc.tensor.matmul(
                    ps_q[:], x2b[:, k : k + 1], wt[:, 0:512],
                    start=(k == 0), stop=(k == 15),
                )
                nc.tensor.matmul(
                    ps_v[:], x2b[:, k : k + 1], wt[:, 512:768],
                    start=(k == 0), stop=(k == 15),
                )
            qkv_row = small.tile([1, 768], dt, tag="qkv")
            nc.scalar.mul(qkv_row[:, 0:256], ps_q[:, 0:256], SCALE)
            nc.scalar.copy(qkv_row[:, 256:512], ps_q[:, 256:512])
            nc.scalar.copy(qkv_row[:, 512:768], ps_v[:])

            # ---- transpose q,k to column layout: qkT[:, 0:2]=q heads, 2:4=k heads
            st4 = stage.tile([4, 128], dt, tag="st4")
            nc.scalar.dma_start(st4[:], qkv_row[:, 0:512])
            qkT_ps = ps.tile([128, 4], dt, tag="b")
            nc.tensor.transpose(qkT_ps[:], st4[:], ident[0:4, 0:4])
            qkT = small.tile([128, 4], bf, tag="qkT")
            nc.vector.tensor_copy(qkT[:], qkT_ps[:])

            # ---- attention scores, directly transposed: each 128-wide K block
            # is the PE stationary operand, q the (N=1) moving operand, so
            # att lands as [t-in-block, block] columns with no SBUF reshape.
            # wTboth[p, h*64 + c] = exp(att_h[c*128 + p])  (bf16 PV columns)
            wTboth = small.tile([128, 128], bf, tag="wTboth")
            esp2 = small.tile([128, 2], dt, tag="esp2")  # per-partition exp sums
            for h in range(HPC):
                kt = stile("key", 8192, f8)
                nc.sync.dma_start(kt[:], keys_r[:, h * 8192 : (h + 1) * 8192])
                att_ps = ps.tile([128, 64], dt, tag="b", name=f"attps{h}")
                for b in range(64):
                    nc.tensor.matmul(
                        att_ps[:, b : b + 1],
                        kt[:, b * 128 : (b + 1) * 128],
                        qkT[:, h : h + 1],
                        start=True, stop=True,
                    )
                nc.scalar.activation(
                    wTboth[:, h * 64 : (h + 1) * 64], att_ps[:], AF.Exp,
                    accum_out=esp2[:, h : h + 1],
                )

            # current-token score per head: exp(q_h . k_h) (SCALE folded into q)
            e_last = small.tile([1, 2], dt, tag="elast")
            for h in range(HPC):
                pal = ps.tile([1, 1], dt, tag="b")
                nc.tensor.matmul(
                    pal[:], qkT[:, h : h + 1], qkT[:, 2 + h : 3 + h],
                    start=True, stop=True,
                )
                nc.scalar.activation(e_last[:, h : h + 1], pal[:], AF.Exp)

            # softmax denominators: cross-partition sum of esp2 + e_last
            dps = ps.tile([1, 2], dt, tag="b")
            nc.tensor.matmul(dps[:], ones_col[:], esp2[:], start=True, stop=True)
            dtmp = small.tile([1, 2], dt, tag="dtmp")
            nc.vector.tensor_add(dtmp[:], dps[:], e_last[:])
            dinv = small.tile([1, 2], dt, tag="dinv")
            nc.vector.reciprocal(dinv[:], dtmp[:])

            # ---- PV on the PE: lhsT = exp column [128, 1] (t-block on
            # partitions), rhs = V block [t, d]; 64 accumulating N=128
            # matmuls per head.
            pv_ps = [
                ps.tile([1, 128], dt, tag="b", name=f"pv_ps{h}") for h in range(HPC)
            ]
            for tt in range(2):
                vt = stile("val", 8192, f8)
                nc.sync.dma_start(vt[:], vals_r[:, tt * 8192 : (tt + 1) * 8192])
                for j in range(32):
                    c = tt * 32 + j
                    for h in range(HPC):
                        nc.tensor.matmul(
                            pv_ps[h][:],
                            wTboth[:, h * 64 + c : h * 64 + c + 1],
                            vt[:, j * 256 + h * 128 : j * 256 + (h + 1) * 128],
                            start=(c == 0), stop=(c == 63),
                        )

            # combine with current-token value, then normalize by the softmax sum
            xa_row = small.tile([1, 256], dt, tag="xa")
            for h in range(HPC):
                sl = slice(h * 128, (h + 1) * 128)
                nc.vector.tensor_scalar_mul(
                    xa_row[:, sl],
                    qkv_row[:, 512 + h * 128 : 512 + (h + 1) * 128],
                    e_last[:, h : h + 1],
                )
                nc.vector.tensor_add(xa_row[:, sl], xa_row[:, sl], pv_ps[h][:])
                nc.vector.tensor_scalar_mul(xa_row[:, sl], xa_row[:, sl], dinv[:, h : h + 1])

            # ---- transpose x_attn to column layout [128, 2], fold wo pre-scale
            st2 = stage.tile([2, 128], dt, tag="st2")
            nc.scalar.dma_start(st2[:], xa_row[:])
            xaT_ps = ps.tile([128, 2], dt, tag="b")
            nc.tensor.transpose(xaT_ps[:], st2[:], ident[0:2, 0:2])
            xaT = small.tile([128, 2], bf, tag="xaT")
            nc.scalar.mul(xaT[:], xaT_ps[:], 1.0 / WS)

            # ---- wo partial: [1, 2048] = 4 PE chains over 2 k-columns
            ar1_in = small.tile([1, 2048], dt, tag="arin", name="ar1_in")
            wot = stile("wo", 4096, f8)
            nc.sync.dma_start(wot[:], wo_r[:])
            wo_pe = [ps.tile([1, 512], dt, tag="b", name=f"wope{n}") for n in range(4)]
            for k in range(2):
                for n in range(4):
                    nc.tensor.matmul(
                        wo_pe[n][:], xaT[:, k : k + 1],
                        wot[:, k * 2048 + n * 512 : k * 2048 + (n + 1) * 512],
                        start=(k == 0), stop=(k == 1),
                    )
            for n in range(4):
                if n % 2 == 0:
                    nc.vector.tensor_copy(ar1_in[:, n * 512 : (n + 1) * 512], wo_pe[n][:])
                else:
                    nc.scalar.copy(ar1_in[:, n * 512 : (n + 1) * 512], wo_pe[n][:])

            def all_reduce(row_sb, idx, name):
                """Sum a [1, 2048] partial across cores; returns PSUM [128, 16].

                AllGather + on-core rank reduction: 16 K=8 matmuls against a
                ones vector sum the 8 gathered rows and transpose into the
                [128, 16] column layout.
                """
                in_d = dram.tile([1, 2048], dt, tag=f"{name}_in")
                out_d = dram.tile([N_CORES, 2048], dt, tag=f"{name}_out")
                nc.scalar.dma_start(in_d[:], row_sb[:])
                nc.gpsimd.collective_compute(
                    "AllGather",
                    mybir.AluOpType.bypass,
                    replica_groups=[list(range(N_CORES))],
                    ins=[in_d.opt()],
                    outs=[out_d.opt()],
                )
                ag_sb = stage.tile(
                    [N_CORES, 2048], dt, tag="ag", name=f"ag_{name}", bufs=1
                )
                nc.scalar.dma_start(ag_sb[:], out_d[:])
                # DVE delay chain of copies feeding dummy matmuls: spreads
                # PE activity across the collective wait so HAM stays at
                # 8/8 and the rank-reduce below runs warm
                wd_prev = wscr
                for i in range(8):
                    wd = stage.tile(
                        [128, 512], bf, tag="wd", name=f"wd_{name}{i}", bufs=2
                    )
                    nc.vector.tensor_copy(wd[:], wd_prev[:])
                    wm = ps.tile([1, 512], dt, tag="wm", bufs=1, name="wm")
                    nc.tensor.matmul(wm[:], ones_bf[:], wd[:], start=True, stop=True)
                    wd_prev = wd
                x_ps = ps.tile([128, 16], dt, tag="b", name=f"xps_{name}")
                for c in range(16):
                    nc.tensor.matmul(
                        x_ps[:, c : c + 1],
                        ag_sb[:, c * 128 : (c + 1) * 128],
                        ones_col[0:N_CORES, :],
                        start=True, stop=True,
                    )
                return x_ps

            x3_ps = all_reduce(ar1_in, 0, "ar1")
            x3 = small.tile([128, 16], dt, tag="x3")
            nc.vector.tensor_add(x3[:], x3_ps[:], x1[:])  # + residual

            # ---- MLP1: h = relu(w1 @ x4), 2 PE chains over 16 k-columns ----
            x4b = rms(x3, "n3", out_dtype=bf)
            ph = [ps.tile([1, 512], dt, tag="b", name=f"ph{n}") for n in range(2)]
            for a in range(2):
                w1t = stile("w1", 8192, bf)
                nc.sync.dma_start(w1t[:], w1_r[:, a * 8192 : (a + 1) * 8192])
                for b in range(8):
                    k = a * 8 + b
                    for n in range(2):
                        nc.tensor.matmul(
                            ph[n][:], x4b[:, k : k + 1],
                            w1t[:, b * 1024 + n * 512 : b * 1024 + (n + 1) * 512],
                            start=(k == 0), stop=(k == 15),
                        )
            h_row = small.tile([1, 1024], dt, tag="hrow")
            for n in range(2):
                nc.scalar.activation(h_row[:, n * 512 : (n + 1) * 512], ph[n][:], AF.Relu)

            st8 = stage.tile([8, 128], dt, tag="st8")
            nc.scalar.dma_start(st8[:], h_row[:])
            hT_ps = ps.tile([128, 8], dt, tag="b")
            nc.tensor.transpose(hT_ps[:], st8[:], ident[0:8, 0:8])
            hT = small.tile([128, 8], bf, tag="hT")
            nc.vector.tensor_copy(hT[:], hT_ps[:])

            # ---- MLP2: [1, 2048] = 4 PE chains over 8 k-columns ----
            ar2_in = small.tile([1, 2048], dt, tag="arin", name="ar2_in")
            pm = [ps.tile([1, 512], dt, tag="b", name=f"pm{n}") for n in range(4)]
            for a in range(2):
                w2t = stile("w2", 8192, bf)
                nc.sync.dma_start(w2t[:], w2_r[:, a * 8192 : (a + 1) * 8192])
                for b in range(4):
                    k = a * 4 + b
                    for n in range(4):
                        nc.tensor.matmul(
                            pm[n][:], hT[:, k : k + 1],
                            w2t[:, b * 2048 + n * 512 : b * 2048 + (n + 1) * 512],
                            start=(k == 0), stop=(k == 7),
                        )
            for n in range(4):
                if n % 2 == 0:
                    nc.vector.tensor_copy(ar2_in[:, n * 512 : (n + 1) * 512], pm[n][:])
                else:
                    nc.scalar.copy(ar2_in[:, n * 512 : (n + 1) * 512], pm[n][:])

            x5_ps = all_reduce(ar2_in, 1, "ar2")
            x5f = small.tile([128, 16], dt, tag="x5f")
            nc.vector.tensor_add(x5f[:], x5_ps[:], x3[:])  # + residual (x3)
            x5b = small.tile([128, 16], bf, tag="x5b")
            nc.scalar.copy(x5b[:], x5f[:])

            # ---- LM head over the vocab shard: 13 k-innermost column blocks,
            # each one contiguous DMA. PE blocks: one 16-matmul PSUM chain.
            # DVE blocks: 16 scalar_tensor_tensor accumulates into an SBUF acc
            # with the cross-partition ones-matmul reduce deferred so the PE
            # never stalls waiting on the DVE.
            def drain(pl, lo, w, cb):
                lr = small.tile([1, 512], dt, tag="lrow", name=f"lr{cb}", bufs=3)
                nc.vector.tensor_copy(lr[:, 0:w], pl[:])
                nc.scalar.dma_start(logits_out[:, lo : lo + w], lr[:, 0:w])

            pending = []  # deferred DVE-block reduces: (acc, lo, w, cb)
            pe_seen = 0
            off16 = 0
            for cb, (lo, w) in enumerate(LM_BLOCKS):
                lt = stile("lm", 16 * w, bf)
                nc.sync.dma_start(lt[:], lm_r[:, off16 : off16 + 16 * w])
                if cb in DVE_BLOCKS:
                    # bf16 accumulator: both DVE ports 16-bit
                    acc = accp.tile([128, w], bf, tag="acc", name=f"acc{cb}")
                    for k in range(16):
                        if k == 0:
                            nc.vector.tensor_scalar_mul(
                                acc[:], lt[:, 0:w], x5f[:, 0:1]
                            )
                        else:
                            nc.vector.scalar_tensor_tensor(
                                acc[:], lt[:, k * w : (k + 1) * w],
                                x5f[:, k : k + 1], acc[:],
                                op0=MUL, op1=ADD,
                            )
                    pending.append((acc, lo, w, cb))
                else:
                    pe_seen += 1
                    if pe_seen in (4, 8) and pending:
                        acc, plo, pw, pcb = pending.pop(0)
                        pr = ps.tile([1, pw], dt, tag="b", name=f"pr{pcb}")
                        nc.tensor.matmul(
                            pr[:], ones_bf[:], acc[:], start=True, stop=True
                        )
                        drain(pr, plo, pw, pcb)
                    pl = ps.tile([1, w], dt, tag="b", name=f"pl{cb}")
                    for k in range(16):
                        nc.tensor.matmul(
                            pl[:], x5b[:, k : k + 1], lt[:, k * w : (k + 1) * w],
                            start=(k == 0), stop=(k == 15),
                        )
                    drain(pl, lo, w, cb)
                off16 += 16 * w
            assert not pending, "deferred DVE reduces must drain before cb12"

            # warmup-collective readback + DCE-keeper. The pin copy below
            # writes the destination tile from x5f first, so the readback
            # DMA (WAW on that tile) cannot be scheduled into any engine
            # FIFO until after AR2 -- otherwise the scheduler hoists it and
            # the ACT queue blocks mid-kernel on the warmup AG.
            warm_back = stage.tile([1, 16], dt, tag="warmb", bufs=1)
            nc.vector.tensor_copy(warm_back[:], x5f[0:1, :])  # ordering pin
            nc.scalar.dma_start(warm_back[:], warm_out[0:1, :])
            klo, kw = LM_BLOCKS[-1]
            kr = small.tile([1, 16], dt, tag="keep")
            nc.vector.scalar_tensor_tensor(
                kr[:], warm_back[:], 0.0, warm_back[:], op0=MUL, op1=MUL
            )
            nc.gpsimd.dma_start(
                logits_out[:, klo + kw - 16 : klo + kw], kr[:],
                accum_op=ADD,
            )

    nc.finalize()
    return nc


def _col16(v):
    """[2048] vector -> [128, 16] column-major layout (e = c*128 + p at [p, c])."""
    return np.ascontiguousarray(v.reshape(16, 128).T)


def _part_major(mT, nblk, blk_rows, width):
    """[nblk*blk_rows, width] -> [blk_rows, nblk*width] partition-major."""
    return np.ascontiguousarray(
        mT.reshape(nblk, blk_rows, width).transpose(1, 0, 2).reshape(blk_rows, nblk * width)
    )


def _to_f8(a):
    import ml_dtypes

    return np.clip(a, -240.0, 240.0).astype(ml_dtypes.float8_e4m3)


def _to_bf(a):
    import ml_dtypes

    return a.astype(ml_dtypes.bfloat16)


def _lm_blocked(shard):
    """[VPC, E] fp32 -> [128, 16*VPC] bf16 with k-innermost 512-col blocks."""
    cols = []
    for lo, w in LM_BLOCKS:
        blk = shard[lo : lo + w].T  # [E, w]
        cols.append(blk.reshape(16, 128, w).transpose(1, 0, 2).reshape(128, 16 * w))
    return _to_bf(np.concatenate(cols, axis=1))


def _prep_in_maps(token_id, pos_id, keys, values, wte, wpe, wq, wk, wv, wo, w1, w2, lm_w):
    f32 = lambda a: np.asarray(a, dtype=np.float32)
    keys, values = f32(keys), f32(values)
    wq, wk, wv, wo, w1, w2, lm_w = map(f32, (wq, wk, wv, wo, w1, w2, lm_w))
    xe_wte = _col16(f32(wte[token_id]))
    xe_wpe = _col16(f32(wpe[pos_id]))
    lm_pad = np.zeros((N_CORES * VPC, E), np.float32)
    lm_pad[:VOCAB] = lm_w

    in_maps = []
    for i in range(N_CORES):
        hs = slice(i * EPC, (i + 1) * EPC)
        wqkv = np.concatenate([wq[hs], wk[hs], wv[hs]], axis=0)  # [768, E]
        in_maps.append(
            {
                "xe_wte": xe_wte,
                "xe_wpe": xe_wpe,
                "wqkv_r": _to_f8(
                    _part_major(np.ascontiguousarray(wqkv.T) * WS, 16, 128, 768)
                ),
                "keys_r": _to_f8(
                    _part_major(np.ascontiguousarray(keys[:, hs].T), 2, 128, 8192)
                ),
                "vals_r": _to_f8(_part_major(values[:, hs], 64, 128, EPC)),
                "wo_r": _to_f8(
                    _part_major(np.ascontiguousarray(wo[:, hs].T) * WS, 2, 128, E)
                ),
                "w1_r": _to_bf(
                    _part_major(
                        np.ascontiguousarray(w1[i * 1024 : (i + 1) * 1024].T),
                        16, 128, 1024,
                    )
                ),
                "w2_r": _to_bf(
                    _part_major(
                        np.ascontiguousarray(w2[:, i * 1024 : (i + 1) * 1024].T),
                        8, 128, E,
                    )
                ),
                "lm_r": _lm_blocked(lm_pad[i * VPC : (i + 1) * VPC]),
            }
        )
    return in_maps


def kernel(**inputs) -> np.ndarray:
    from concourse.bass_utils import run_bass_kernel_spmd

    token_id = int(inputs["token_id"])
    pos_id = int(inputs["pos_id"])
    in_maps = _prep_in_maps(
        token_id,
        pos_id,
        inputs["keys"],
        inputs["values"],
        inputs["wte"],
        inputs["wpe"],
        inputs["wq"],
        inputs["wk"],
        inputs["wv"],
        inputs["wo"],
        inputs["w1"],
        inputs["w2"],
        inputs["lm_w"],
    )
    if "nc" not in _CACHE:
        _CACHE["nc"] = _build_nc()
    nc = _CACHE["nc"]
    res = run_bass_kernel_spmd(
        nc,
        in_maps,
        core_ids=list(range(N_CORES)),
        trace=TRACE,
        trace_cores=[0] if TRACE else None,
    )
    _CACHE["last_result"] = res
    logits = np.concatenate([r["logits"][0] for r in res.results])[:VOCAB]
    return np.ascontiguousarray(logits.astype(np.float32))


# revision 42
# speedup vs baseline: 1.1538x; 1.0489x over previous
"""Tensor-parallel MiniGPT single-token decode step on 8 Trainium2 NeuronCores.

Sharding (per core i of 8):
  - attention: heads 2i, 2i+1 (head_dim 128 -> cols i*256:(i+1)*256 of E=2048);
    wq/wk/wv row-sharded, wo column-sharded, KV cache column-sharded by head.
  - MLP: w1 row-sharded (1024 rows/core), w2 column-sharded.
  - LM head: vocab-sharded (50257 padded to 8*6283=50264 rows).
  - Two 8KB AllGathers + on-core rank reduction combine the wo- and w2-
    partial sums; logits are gathered on the host.

Memory-bound regime: all streamed weights are narrow on the wire.
  - fp8 e4m3: wqkv, K cache, V cache, wo  (attention output is ~1% of the
    residual stream, so 3.6% quantization RMS there is ~4e-4 end to end).
    wqkv/wo are pre-scaled x1024 on the host so sigma=0.02 weights land in
    e4m3's normal range; the 1/1024 is folded into the bf16 activation.
  - bf16: w1, w2, lm_head (these feed logits directly; bf16 keeps ~1e-3).

Engine plan: every matvec contraction runs on the PE (bf16/fp8 moving
operand streams 128 elements/cycle); attention scores are computed
directly in transposed [t-in-block, block] form by loading each K block
as the PE stationary operand, which kills the row->column SBUF-DMA
transpose pipeline. PV accumulates per head with N=128 matmuls
(lhsT = one exp column per t-block). The lm_head shard is k-innermost 512-column
blocks (one contiguous 2MB DMA = one PSUM chain); four of the 13 blocks
run on the DVE (scalar_tensor_tensor accumulate + a deferred ones-matmul
partition reduce) so the post-AllReduce tail is not PE-serial. One shared
9-slot SBUF stream ring lets the lm stream run ahead ~10MB while the
AllGathers are in flight, keeping the DMA queues busy end to end.
"""

import numpy as np

N_CORES = 8
E = 2048
HPC = 2  # heads per core
EPC = HPC * 128  # 256
T = 8192
VOCAB = 50257
VPC = 6283  # padded vocab rows per core (8 * 6283 = 50264)
SCALE = float(1.0 / np.sqrt(128.0))
EPS = 1e-5
WS = 1024.0  # fp8 pre-scale for wqkv / wo

# lm_head chain blocks: (col offset within shard, width)
LM_BLOCKS = [(i * 512, 512) for i in range(12)] + [(6144, VPC - 6144)]
DVE_BLOCKS = (0, 2)  # lm blocks accumulated on the DVE instead of PE

_CACHE = {}
TRACE = False


def _build_nc():
    import concourse.bacc as bacc
    import concourse.mybir as mybir
    import concourse.tile as tile
    from concourse.masks import make_identity

    AF = mybir.ActivationFunctionType
    MUL = mybir.AluOpType.mult
    ADD = mybir.AluOpType.add
    dt = mybir.dt.float32
    bf = mybir.dt.bfloat16
    f8 = mybir.dt.float8e4

    nc = bacc.Bacc(
        "TRN2", target_bir_lowering=False, debug=False, num_devices=N_CORES
    )

    xe_wte = nc.declare_dram_parameter("xe_wte", [128, 16], dt, isOutput=False)
    xe_wpe = nc.declare_dram_parameter("xe_wpe", [128, 16], dt, isOutput=False)
    wqkv_r = nc.declare_dram_parameter("wqkv_r", [128, 16 * 768], f8, isOutput=False)
    keys_r = nc.declare_dram_parameter("keys_r", [128, 2 * 8192], f8, isOutput=False)
    vals_r = nc.declare_dram_parameter("vals_r", [128, 64 * 256], f8, isOutput=False)
    wo_r = nc.declare_dram_parameter("wo_r", [128, 2 * 2048], f8, isOutput=False)
    w1_r = nc.declare_dram_parameter("w1_r", [128, 16 * 1024], bf, isOutput=False)
    w2_r = nc.declare_dram_parameter("w2_r", [128, 8 * 2048], bf, isOutput=False)
    lm_r = nc.declare_dram_parameter("lm_r", [128, 16 * VPC], bf, isOutput=False)
    logits_out = nc.declare_dram_parameter("logits", [1, VPC], dt, isOutput=True)

    with tile.TileContext(nc) as tc:
        with (
            tc.tile_pool(name="const", bufs=1) as const,
            tc.tile_pool(name="small", bufs=1) as small,
            tc.tile_pool(name="stage", bufs=2) as stage,
            tc.tile_pool(name="ps", bufs=7, space="PSUM") as ps,
            tc.tile_pool(name="dram", bufs=1, space="DRAM") as dram,
            tc.tile_pool(name="stream", bufs=9) as stream,
            tc.tile_pool(name="acc", bufs=2) as accp,
        ):
            _snum = [0]

            def stile(label, width, dtype):
                # one shared ring of slots; slot size = max tile = 16KB/part
                _snum[0] += 1
                return stream.tile(
                    [128, width], dtype, tag="s", name=f"s{_snum[0]}_{label}"
                )

            # Warm up the collectives path first: a NEFF's first collective
            # pays a large ncfw init (~60-80us observed) that would otherwise
            # land on AR1's critical path. Its output readback + DCE-keeper
            # are emitted at the very END of the program so no engine FIFO
            # ever blocks on this collective's completion.
            warm_in = dram.tile([1, 16], dt, tag="warm_in")
            warm_out = dram.tile([N_CORES, 16], dt, tag="warm_out")
            warm_sb = stage.tile([1, 16], dt, tag="warm", bufs=1)
            nc.vector.memset(warm_sb[:], 0.0)
            nc.gpsimd.dma_start(warm_in[:], warm_sb[:])
            nc.gpsimd.collective_compute(
                "AllGather",
                mybir.AluOpType.bypass,
                replica_groups=[list(range(N_CORES))],
                ins=[warm_in.opt()],
                outs=[warm_out.opt()],
            )

            # ---- embedding row loads lead the weight-stream queue ----
            xw = stage.tile([128, 16], dt, tag="xw")
            nc.sync.dma_start(xw[:], xe_wte[:])
            xp = stage.tile([128, 16], dt, tag="xp")
            nc.sync.dma_start(xp[:], xe_wpe[:])

            ones_col = const.tile([128, 1], dt)
            nc.vector.memset(ones_col[:], 1.0)
            ones_row = const.tile([1, 128], dt)
            nc.vector.memset(ones_row[:], 1.0)
            ones_row_ws = const.tile([1, 128], dt)
            nc.vector.memset(ones_row_ws[:], 1.0 / WS)
            ident = const.tile([16, 16], dt)
            make_identity(nc, ident[:])
            eps_c = const.tile([1, 1], dt)
            nc.vector.memset(eps_c[:], EPS)
            ones_bf = const.tile([128, 1], bf)
            nc.vector.memset(ones_bf[:], 1.0)

            wscr = const.tile([128, 512], bf)
            nc.vector.memset(wscr[:], 0.25)

            def warm(n):
                # keep-the-HAM-warm dummies: cheap bf16 matmuls on resident data
                for _ in range(n):
                    wm = ps.tile([1, 512], dt, tag="wm", bufs=1, name="wm")
                    nc.tensor.matmul(
                        wm[:], ones_bf[:], wscr[:], start=True, stop=True
                    )

            def rms(xt, name, out_dtype=dt, rowc=None):
                """x * rsqrt(mean(x^2) + eps) for x in [128, 16] column layout.

                rowc: [1, 128] broadcast row; its value multiplies the
                rsqrt scale (used to fold the fp8 weight pre-scale in).
                """
                sq = small.tile([128, 16], dt, tag=f"sq_{name}")
                ssum = small.tile([128, 1], dt, tag=f"ss_{name}")
                nc.scalar.activation(sq[:], xt[:], AF.Square, accum_out=ssum[:])
                tot = ps.tile([1, 1], dt, tag="b")
                nc.tensor.matmul(tot[:], ssum[:], ones_col[:], start=True, stop=True)
                std = small.tile([1, 1], dt, tag=f"std_{name}")
                nc.scalar.activation(
                    std[:], tot[:], AF.Sqrt, bias=eps_c[:], scale=1.0 / float(E)
                )
                inv = small.tile([1, 1], dt, tag=f"inv_{name}")
                nc.vector.reciprocal(inv[:], std[:])
                invb_ps = ps.tile([128, 1], dt, tag="b")
                nc.tensor.matmul(
                    invb_ps[:], rowc if rowc is not None else ones_row[:],
                    inv[:], start=True, stop=True,
                )
                xn = small.tile([128, 16], out_dtype, tag=f"xn_{name}")
                nc.vector.tensor_scalar_mul(xn[:], xt[:], invb_ps[:])
                return xn

            x0 = small.tile([128, 16], dt, tag="x0")
            nc.vector.tensor_add(x0[:], xw[:], xp[:])
            x1 = rms(x0, "n1")  # residual input (fp32)
            # second rms emits bf16 with the wqkv fp8 pre-scale folded in
            x2b = rms(x1, "n2", out_dtype=bf, rowc=ones_row_ws[:])

            # ---- qkv projection: [1, 768] row (q 0:256 | k 256:512 | v 512:768)
            wqkv_t = stile("qkv", 16 * 768, f8)
            nc.sync.dma_start(wqkv_t[:], wqkv_r[:])
            ps_q = ps.tile([1, 512], dt, tag="b", name="ps_q")
            ps_v = ps.tile([1, 256], dt, tag="b", name="ps_v")
            for k in range(16):
                wt = wqkv_t[:, k * 768 : (k + 1) * 768]
                nc.tensor.matmul(
                    ps_q[:], x2b[:, k : k + 1], wt[:, 0:512],
                    start=(k == 0), stop=(k == 15),
                )
                nc.tensor.matmul(
                    ps_v[:], x2b[:, k : k + 1], wt[:, 512:768],
                    start=(k == 0), stop=(k == 15),
                )
            qkv_row = small.tile([1, 768], dt, tag="qkv")
            nc.scalar.mul(qkv_row[:, 0:256], ps_q[:, 0:256], SCALE)
            nc.scalar.copy(qkv_row[:, 256:512], ps_q[:, 256:512])
            nc.scalar.copy(qkv_row[:, 512:768], ps_v[:])

            # ---- transpose q,k to column layout: qkT[:, 0:2]=q heads, 2:4=k heads
            st4 = stage.tile([4, 128], dt, tag="st4")
            nc.scalar.dma_start(st4[:], qkv_row[:, 0:512])
            qkT_ps = ps.tile([128, 4], dt, tag="b")
            nc.tensor.transpose(qkT_ps[:], st4[:], ident[0:4, 0:4])
            qkT = small.tile([128, 4], bf, tag="qkT")
            nc.vector.tensor_copy(qkT[:], qkT_ps[:])

            # ---- attention scores, directly transposed: each 128-wide K block
            # is the PE stationary operand, q the (N=1) moving operand, so
            # att lands as [t-in-block, block] columns with no SBUF reshape.
            # wTboth[p, h*64 + c] = exp(att_h[c*128 + p])  (bf16 PV columns)
            wTboth = small.tile([128, 128], bf, tag="wTboth")
            esp2 = small.tile([128, 2], dt, tag="esp2")  # per-partition exp sums
            for h in range(HPC):
                kt = stile("key", 8192, f8)
                nc.sync.dma_start(kt[:], keys_r[:, h * 8192 : (h + 1) * 8192])
                att_ps = ps.tile([128, 64], dt, tag="b", name=f"attps{h}")
                for b in range(64):
                    nc.tensor.matmul(
                        att_ps[:, b : b + 1],
                        kt[:, b * 128 : (b + 1) * 128],
                        qkT[:, h : h + 1],
                        start=True, stop=True,
                    )
                nc.scalar.activation(
                    wTboth[:, h * 64 : (h + 1) * 64], att_ps[:], AF.Exp,
                    accum_out=esp2[:, h : h + 1],
                )

            # current-token score per head: exp(q_h . k_h) (SCALE folded into q)
            e_last = small.tile([1, 2], dt, tag="elast")
            for h in range(HPC):
                pal = ps.tile([1, 1], dt, tag="b")
                nc.tensor.matmul(
                    pal[:], qkT[:, h : h + 1], qkT[:, 2 + h : 3 + h],
                    start=True, stop=True,
                )
                nc.scalar.activation(e_last[:, h : h + 1], pal[:], AF.Exp)

            # softmax denominators: cross-partition sum of esp2 + e_last
            dps = ps.tile([1, 2], dt, tag="b")
            nc.tensor.matmul(dps[:], ones_col[:], esp2[:], start=True, stop=True)
            dtmp = small.tile([1, 2], dt, tag="dtmp")
            nc.vector.tensor_add(dtmp[:], dps[:], e_last[:])
            dinv = small.tile([1, 2], dt, tag="dinv")
            nc.vector.reciprocal(dinv[:], dtmp[:])

            # ---- PV on the PE: lhsT = exp column [128, 1] (t-block on
            # partitions), rhs = V block [t, d]; 64 accumulating N=128
            # matmuls per head.
            pv_ps = [
                ps.tile([1, 128], dt, tag="b", name=f"pv_ps{h}") for h in range(HPC)
            ]
            for tt in range(2):
                vt = stile("val", 8192, f8)
                nc.sync.dma_start(vt[:], vals_r[:, tt * 8192 : (tt + 1) * 8192])
                for j in range(32):
                    c = tt * 32 + j
                    for h in range(HPC):
                        nc.tensor.matmul(
                            pv_ps[h][:],
                            wTboth[:, h * 64 + c : h * 64 + c + 1],
                            vt[:, j * 256 + h * 128 : j * 256 + (h + 1) * 128],
                            start=(c == 0), stop=(c == 63),
                        )

            # combine with current-token value, then normalize by the softmax sum
            xa_row = small.tile([1, 256], dt, tag="xa")
            for h in range(HPC):
                sl = slice(h * 128, (h + 1) * 128)
                nc.vector.tensor_scalar_mul(
                    xa_row[:, sl],
                    qkv_row[:, 512 + h * 128 : 512 + (h + 1) * 128],
                    e_last[:, h : h + 1],
                )
                nc.vector.tensor_add(xa_row[:, sl], xa_row[:, sl], pv_ps[h][:])
                nc.vector.tensor_scalar_mul(xa_row[:, sl], xa_row[:, sl], dinv[:, h : h + 1])

            # ---- transpose x_attn to column layout [128, 2], fold wo pre-scale
            st2 = stage.tile([2, 128], dt, tag="st2")
            nc.scalar.dma_start(st2[:], xa_row[:])
            xaT_ps = ps.tile([128, 2], dt, tag="b")
            nc.tensor.transpose(xaT_ps[:], st2[:], ident[0:2, 0:2])
            xaT = small.tile([128, 2], bf, tag="xaT")
            nc.scalar.mul(xaT[:], xaT_ps[:], 1.0 / WS)

            # ---- wo partial: [1, 2048] = 4 PE chains over 2 k-columns
            ar1_in = small.tile([1, 2048], dt, tag="arin", name="ar1_in")
            wot = stile("wo", 4096, f8)
            nc.sync.dma_start(wot[:], wo_r[:])
            wo_pe = [ps.tile([1, 512], dt, tag="b", name=f"wope{n}") for n in range(4)]
            for k in range(2):
                for n in range(4):
                    nc.tensor.matmul(
                        wo_pe[n][:], xaT[:, k : k + 1],
                        wot[:, k * 2048 + n * 512 : k * 2048 + (n + 1) * 512],
                        start=(k == 0), stop=(k == 1),
                    )
            for n in range(4):
                if n % 2 == 0:
                    nc.vector.tensor_copy(ar1_in[:, n * 512 : (n + 1) * 512], wo_pe[n][:])
                else:
                    nc.scalar.copy(ar1_in[:, n * 512 : (n + 1) * 512], wo_pe[n][:])

            def all_reduce(row_sb, idx, name):
                """Sum a [1, 2048] partial across cores; returns PSUM [128, 16].

                AllGather + on-core rank reduction: 16 K=8 matmuls against a
                ones vector sum the 8 gathered rows and transpose into the
                [128, 16] column layout.
                """
                in_d = dram.tile([1, 2048], dt, tag=f"{name}_in")
                out_d = dram.tile([N_CORES, 2048], dt, tag=f"{name}_out")
                nc.scalar.dma_start(in_d[:], row_sb[:])
                nc.gpsimd.collective_compute(
                    "AllGather",
                    mybir.AluOpType.bypass,
                    replica_groups=[list(range(N_CORES))],
                    ins=[in_d.opt()],
                    outs=[out_d.opt()],
                )
                ag_sb = stage.tile(
                    [N_CORES, 2048], dt, tag="ag", name=f"ag_{name}", bufs=1
                )
                nc.scalar.dma_start(ag_sb[:], out_d[:])
                if name == "ar2":
                    # DVE delay chain feeding dummy matmuls: PE activity is
                    # spread over this (short) collective wait so HAM stays
                    # warm and the rank-reduce runs at full clock. (For ar1
                    # the wait is far longer than any chain could span.)
                    wd_prev = wscr
                    for i in range(10):
                        wd = stage.tile(
                            [128, 512], bf, tag="wd", name=f"wd_{name}{i}", bufs=2
                        )
                        nc.vector.tensor_copy(wd[:], wd_prev[:])
                        wm = ps.tile([1, 512], dt, tag="wm", bufs=1, name="wm")
                        nc.tensor.matmul(
                            wm[:], ones_bf[:], wd[:], start=True, stop=True
                        )
                        wd_prev = wd
                x_ps = ps.tile([128, 16], dt, tag="b", name=f"xps_{name}")
                for c in range(16):
                    nc.tensor.matmul(
                        x_ps[:, c : c + 1],
                        ag_sb[:, c * 128 : (c + 1) * 128],
                        ones_col[0:N_CORES, :],
                        start=True, stop=True,
                    )
                return x_ps

            x3_ps = all_reduce(ar1_in, 0, "ar1")
            x3 = small.tile([128, 16], dt, tag="x3")
            nc.vector.tensor_add(x3[:], x3_ps[:], x1[:])  # + residual

            # ---- MLP1: h = relu(w1 @ x4), 2 PE chains over 16 k-columns ----
            x4b = rms(x3, "n3", out_dtype=bf)
            ph = [ps.tile([1, 512], dt, tag="b", name=f"ph{n}") for n in range(2)]
            for a in range(2):
                w1t = stile("w1", 8192, bf)
                nc.sync.dma_start(w1t[:], w1_r[:, a * 8192 : (a + 1) * 8192])
                for b in range(8):
                    k = a * 8 + b
                    for n in range(2):
                        nc.tensor.matmul(
                            ph[n][:], x4b[:, k : k + 1],
                            w1t[:, b * 1024 + n * 512 : b * 1024 + (n + 1) * 512],
                            start=(k == 0), stop=(k == 15),
                        )
            # relu + transpose per 512-col half so MLP2's first k-group can
            # start while the second half is still in its SBUF-DMA hop
            h_row = small.tile([1, 1024], dt, tag="hrow")
            hT = []
            for n in range(2):
                nc.scalar.activation(h_row[:, n * 512 : (n + 1) * 512], ph[n][:], AF.Relu)
                st4h = stage.tile([4, 128], dt, tag="st4h", name=f"st4h{n}", bufs=2)
                nc.scalar.dma_start(st4h[:], h_row[:, n * 512 : (n + 1) * 512])
                hT_ps = ps.tile([128, 4], dt, tag="b", name=f"hTps{n}")
                nc.tensor.transpose(hT_ps[:], st4h[:], ident[0:4, 0:4])
                ht = small.tile([128, 4], bf, tag="hT", name=f"hT{n}", bufs=2)
                nc.vector.tensor_copy(ht[:], hT_ps[:])
                hT.append(ht)

            # ---- MLP2: [1, 2048] = 4 PE chains over 8 k-columns ----
            ar2_in = small.tile([1, 2048], dt, tag="arin", name="ar2_in")
            pm = [ps.tile([1, 512], dt, tag="b", name=f"pm{n}") for n in range(4)]
            for a in range(2):
                w2t = stile("w2", 8192, bf)
                nc.sync.dma_start(w2t[:], w2_r[:, a * 8192 : (a + 1) * 8192])
                for b in range(4):
                    k = a * 4 + b
                    for n in range(4):
                        nc.tensor.matmul(
                            pm[n][:], hT[k // 4][:, k % 4 : k % 4 + 1],
                            w2t[:, b * 2048 + n * 512 : b * 2048 + (n + 1) * 512],
                            start=(k == 0), stop=(k == 7),
                        )
            for n in range(4):
                if n % 2 == 0:
                    nc.vector.tensor_copy(ar2_in[:, n * 512 : (n + 1) * 512], pm[n][:])
                else:
                    nc.scalar.copy(ar2_in[:, n * 512 : (n + 1) * 512], pm[n][:])

            x5_ps = all_reduce(ar2_in, 1, "ar2")
            x5f = small.tile([128, 16], dt, tag="x5f")
            nc.vector.tensor_add(x5f[:], x5_ps[:], x3[:])  # + residual (x3)
            x5b = small.tile([128, 16], bf, tag="x5b")
            nc.scalar.copy(x5b[:], x5f[:])

            # ---- LM head over the vocab shard: 13 k-innermost column blocks,
            # each one contiguous DMA. PE blocks: one 16-matmul PSUM chain.
            # DVE blocks: 16 scalar_tensor_tensor accumulates into an SBUF acc
            # with the cross-partition ones-matmul reduce deferred so the PE
            # never stalls waiting on the DVE.
            def drain(pl, lo, w, cb):
                lr = small.tile([1, 512], dt, tag="lrow", name=f"lr{cb}", bufs=3)
                nc.vector.tensor_copy(lr[:, 0:w], pl[:])
                nc.scalar.dma_start(logits_out[:, lo : lo + w], lr[:, 0:w])

            pending = []  # deferred DVE-block reduces: (acc, lo, w, cb)
            pe_seen = 0
            off16 = 0
            for cb, (lo, w) in enumerate(LM_BLOCKS):
                lt = stile("lm", 16 * w, bf)
                nc.sync.dma_start(lt[:], lm_r[:, off16 : off16 + 16 * w])
                if cb in DVE_BLOCKS:
                    # bf16 accumulator: both DVE ports 16-bit
                    acc = accp.tile([128, w], bf, tag="acc", name=f"acc{cb}")
                    for k in range(16):
                        if k == 0:
                            nc.vector.tensor_scalar_mul(
                                acc[:], lt[:, 0:w], x5f[:, 0:1]
                            )
                        else:
                            nc.vector.scalar_tensor_tensor(
                                acc[:], lt[:, k * w : (k + 1) * w],
                                x5f[:, k : k + 1], acc[:],
                                op0=MUL, op1=ADD,
                            )
                    pending.append((acc, lo, w, cb))
                else:
                    pe_seen += 1
                    if pe_seen in (4, 8) and pending:
                        acc, plo, pw, pcb = pending.pop(0)
                        pr = ps.tile([1, pw], dt, tag="b", name=f"pr{pcb}")
                        nc.tensor.matmul(
                            pr[:], ones_bf[:], acc[:], start=True, stop=True
                        )
                        drain(pr, plo, pw, pcb)
                    pl = ps.tile([1, w], dt, tag="b", name=f"pl{cb}")
                    for k in range(16):
                        nc.tensor.matmul(
                            pl[:], x5b[:, k : k + 1], lt[:, k * w : (k + 1) * w],
                            start=(k == 0), stop=(k == 15),
                        )
                    drain(pl, lo, w, cb)
                off16 += 16 * w
            assert not pending, "deferred DVE reduces must drain before cb12"

            # warmup-collective readback + DCE-keeper. The pin copy below
            # writes the destination tile from x5f first, so the readback
            # DMA (WAW on that tile) cannot be scheduled into any engine
            # FIFO until after AR2 -- otherwise the scheduler hoists it and
            # the ACT queue blocks mid-kernel on the warmup AG.
            warm_back = stage.tile([1, 16], dt, tag="warmb", bufs=1)
            nc.vector.tensor_copy(warm_back[:], x5f[0:1, :])  # ordering pin
            nc.scalar.dma_start(warm_back[:], warm_out[0:1, :])
            klo, kw = LM_BLOCKS[-1]
            kr = small.tile([1, 16], dt, tag="keep")
            nc.vector.scalar_tensor_tensor(
                kr[:], warm_back[:], 0.0, warm_back[:], op0=MUL, op1=MUL
            )
            nc.gpsimd.dma_start(
                logits_out[:, klo + kw - 16 : klo + kw], kr[:],
                accum_op=ADD,
            )

    nc.finalize()
    return nc


def _col16(v):
    """[2048] vector -> [128, 16] column-major layout (e = c*128 + p at [p, c])."""
    return np.ascontiguousarray(v.reshape(16, 128).T)


def _part_major(mT, nblk, blk_rows, width):
    """[nblk*blk_rows, width] -> [blk_rows, nblk*width] partition-major."""
    return np.ascontiguousarray(
        mT.reshape(nblk, blk_rows, width).transpose(1, 0, 2).reshape(blk_rows, nblk * width)
    )


def _to_f8(a):
    import ml_dtypes

    return np.clip(a, -240.0, 240.0).astype(ml_dtypes.float8_e4m3)


def _to_bf(a):
    import ml_dtypes

    return a.astype(ml_dtypes.bfloat16)


def _lm_blocked(shard):
    """[VPC, E] fp32 -> [128, 16*VPC] bf16 with k-innermost 512-col blocks."""
    cols = []
    for lo, w in LM_BLOCKS:
        blk = shard[lo : lo + w].T  # [E, w]
        cols.append(blk.reshape(16, 128, w).transpose(1, 0, 2).reshape(128, 16 * w))
    return _to_bf(np.concatenate(cols, axis=1))


def _prep_in_maps(token_id, pos_id, keys, values, wte, wpe, wq, wk, wv, wo, w1, w2, lm_w):
    f32 = lambda a: np.asarray(a, dtype=np.float32)
    keys, values = f32(keys), f32(values)
    wq, wk, wv, wo, w1, w2, lm_w = map(f32, (wq, wk, wv, wo, w1, w2, lm_w))
    xe_wte = _col16(f32(wte[token_id]))
    xe_wpe = _col16(f32(wpe[pos_id]))
    lm_pad = np.zeros((N_CORES * VPC, E), np.float32)
    lm_pad[:VOCAB] = lm_w

    in_maps = []
    for i in range(N_CORES):
        hs = slice(i * EPC, (i + 1) * EPC)
        wqkv = np.concatenate([wq[hs], wk[hs], wv[hs]], axis=0)  # [768, E]
        in_maps.append(
            {
                "xe_wte": xe_wte,
                "xe_wpe": xe_wpe,
                "wqkv_r": _to_f8(
                    _part_major(np.ascontiguousarray(wqkv.T) * WS, 16, 128, 768)
                ),
                "keys_r": _to_f8(
                    _part_major(np.ascontiguousarray(keys[:, hs].T), 2, 128, 8192)
                ),
                "vals_r": _to_f8(_part_major(values[:, hs], 64, 128, EPC)),
                "wo_r": _to_f8(
                    _part_major(np.ascontiguousarray(wo[:, hs].T) * WS, 2, 128, E)
                ),
                "w1_r": _to_bf(
                    _part_major(
                        np.ascontiguousarray(w1[i * 1024 : (i + 1) * 1024].T),
                        16, 128, 1024,
                    )
                ),
                "w2_r": _to_bf(
                    _part_major(
                        np.ascontiguousarray(w2[:, i * 1024 : (i + 1) * 1024].T),
                        8, 128, E,
                    )
                ),
                "lm_r": _lm_blocked(lm_pad[i * VPC : (i + 1) * VPC]),
            }
        )
    return in_maps


def kernel(**inputs) -> np.ndarray:
    from concourse.bass_utils import run_bass_kernel_spmd

    token_id = int(inputs["token_id"])
    pos_id = int(inputs["pos_id"])
    in_maps = _prep_in_maps(
        token_id,
        pos_id,
        inputs["keys"],
        inputs["values"],
        inputs["wte"],
        inputs["wpe"],
        inputs["wq"],
        inputs["wk"],
        inputs["wv"],
        inputs["wo"],
        inputs["w1"],
        inputs["w2"],
        inputs["lm_w"],
    )
    if "nc" not in _CACHE:
        _CACHE["nc"] = _build_nc()
    nc = _CACHE["nc"]
    res = run_bass_kernel_spmd(
        nc,
        in_maps,
        core_ids=list(range(N_CORES)),
        trace=TRACE,
        trace_cores=[0] if TRACE else None,
    )
    _CACHE["last_result"] = res
    logits = np.concatenate([r["logits"][0] for r in res.results])[:VOCAB]
    return np.ascontiguousarray(logits.astype(np.float32))


# revision 43
# speedup vs baseline: 1.2085x; 1.0474x over previous
"""Tensor-parallel MiniGPT single-token decode step on 8 Trainium2 NeuronCores.

Sharding (per core i of 8):
  - attention: heads 2i, 2i+1 (head_dim 128 -> cols i*256:(i+1)*256 of E=2048);
    wq/wk/wv row-sharded, wo column-sharded, KV cache column-sharded by head.
  - MLP: w1 row-sharded (1024 rows/core), w2 column-sharded.
  - LM head: vocab-sharded (50257 padded to 8*6283=50264 rows).
  - Two 8KB AllGathers + on-core rank reduction combine the wo- and w2-
    partial sums; logits are gathered on the host.

Memory-bound regime: all streamed weights are narrow on the wire.
  - fp8 e4m3: wqkv, K cache, V cache, wo  (attention output is ~1% of the
    residual stream, so 3.6% quantization RMS there is ~4e-4 end to end).
    wqkv/wo are pre-scaled x1024 on the host so sigma=0.02 weights land in
    e4m3's normal range; the 1/1024 is folded into the bf16 activation.
  - bf16: w1, w2, lm_head (these feed logits directly; bf16 keeps ~1e-3).

Engine plan: every matvec contraction runs on the PE (bf16/fp8 moving
operand streams 128 elements/cycle); attention scores are computed
directly in transposed [t-in-block, block] form by loading each K block
as the PE stationary operand, which kills the row->column SBUF-DMA
transpose pipeline. PV accumulates per head with N=128 matmuls
(lhsT = one exp column per t-block). The lm_head shard is k-innermost 512-column
blocks (one contiguous 2MB DMA = one PSUM chain); four of the 13 blocks
run on the DVE (scalar_tensor_tensor accumulate + a deferred ones-matmul
partition reduce) so the post-AllReduce tail is not PE-serial. One shared
9-slot SBUF stream ring lets the lm stream run ahead ~10MB while the
AllGathers are in flight, keeping the DMA queues busy end to end.
"""

import numpy as np

N_CORES = 8
E = 2048
HPC = 2  # heads per core
EPC = HPC * 128  # 256
T = 8192
VOCAB = 50257
VPC = 6283  # padded vocab rows per core (8 * 6283 = 50264)
SCALE = float(1.0 / np.sqrt(128.0))
EPS = 1e-5
WS = 1024.0  # fp8 pre-scale for wqkv / wo

# lm_head chain blocks: (col offset within shard, width)
LM_BLOCKS = [(i * 512, 512) for i in range(12)] + [(6144, VPC - 6144)]
DVE_BLOCKS = (0, 2)  # lm blocks accumulated on the DVE instead of PE

_CACHE = {}
TRACE = False


def _build_nc():
    import concourse.bacc as bacc
    import concourse.mybir as mybir
    import concourse.tile as tile
    from concourse.masks import make_identity

    AF = mybir.ActivationFunctionType
    MUL = mybir.AluOpType.mult
    ADD = mybir.AluOpType.add
    dt = mybir.dt.float32
    bf = mybir.dt.bfloat16
    f8 = mybir.dt.float8e4

    nc = bacc.Bacc(
        "TRN2", target_bir_lowering=False, debug=False, num_devices=N_CORES
    )

    xe_wte = nc.declare_dram_parameter("xe_wte", [128, 16], dt, isOutput=False)
    xe_wpe = nc.declare_dram_parameter("xe_wpe", [128, 16], dt, isOutput=False)
    wqkv_r = nc.declare_dram_parameter("wqkv_r", [128, 16 * 768], f8, isOutput=False)
    keys_r = nc.declare_dram_parameter("keys_r", [128, 2 * 8192], f8, isOutput=False)
    vals_r = nc.declare_dram_parameter("vals_r", [128, 64 * 256], f8, isOutput=False)
    wo_r = nc.declare_dram_parameter("wo_r", [128, 2 * 2048], f8, isOutput=False)
    w1_r = nc.declare_dram_parameter("w1_r", [128, 16 * 1024], bf, isOutput=False)
    w2_r = nc.declare_dram_parameter("w2_r", [128, 8 * 2048], bf, isOutput=False)
    lm_r = nc.declare_dram_parameter("lm_r", [128, 16 * VPC], bf, isOutput=False)
    logits_out = nc.declare_dram_parameter("logits", [1, VPC], dt, isOutput=True)

    with tile.TileContext(nc) as tc:
        with (
            tc.tile_pool(name="const", bufs=1) as const,
            tc.tile_pool(name="small", bufs=1) as small,
            tc.tile_pool(name="stage", bufs=2) as stage,
            tc.tile_pool(name="ps", bufs=7, space="PSUM") as ps,
            tc.tile_pool(name="dram", bufs=1, space="DRAM") as dram,
            tc.tile_pool(name="stream", bufs=9) as stream,
            tc.tile_pool(name="acc", bufs=2) as accp,
        ):
            _snum = [0]

            def stile(label, width, dtype):
                # one shared ring of slots; slot size = max tile = 16KB/part
                _snum[0] += 1
                return stream.tile(
                    [128, width], dtype, tag="s", name=f"s{_snum[0]}_{label}"
                )

            # Warm up the collectives path first: a NEFF's first collective
            # pays a large ncfw init (~60-80us observed) that would otherwise
            # land on AR1's critical path. Its output readback + DCE-keeper
            # are emitted at the very END of the program so no engine FIFO
            # ever blocks on this collective's completion.
            warm_in = dram.tile([1, 16], dt, tag="warm_in")
            warm_out = dram.tile([N_CORES, 16], dt, tag="warm_out")
            warm_sb = stage.tile([1, 16], dt, tag="warm", bufs=1)
            nc.vector.memset(warm_sb[:], 0.0)
            nc.gpsimd.dma_start(warm_in[:], warm_sb[:])
            nc.gpsimd.collective_compute(
                "AllGather",
                mybir.AluOpType.bypass,
                replica_groups=[list(range(N_CORES))],
                ins=[warm_in.opt()],
                outs=[warm_out.opt()],
            )

            # ---- embedding row loads lead the weight-stream queue ----
            xw = stage.tile([128, 16], dt, tag="xw")
            nc.sync.dma_start(xw[:], xe_wte[:])
            xp = stage.tile([128, 16], dt, tag="xp")
            nc.sync.dma_start(xp[:], xe_wpe[:])

            ones_col = const.tile([128, 1], dt)
            nc.vector.memset(ones_col[:], 1.0)
            ones_row = const.tile([1, 128], dt)
            nc.vector.memset(ones_row[:], 1.0)
            ones_row_ws = const.tile([1, 128], dt)
            nc.vector.memset(ones_row_ws[:], 1.0 / WS)
            ident = const.tile([16, 16], dt)
            make_identity(nc, ident[:])
            eps_c = const.tile([1, 1], dt)
            nc.vector.memset(eps_c[:], EPS)
            ones_bf = const.tile([128, 1], bf)
            nc.vector.memset(ones_bf[:], 1.0)

            wscr = const.tile([128, 512], bf)
            nc.vector.memset(wscr[:], 0.25)

            def warm(n):
                # keep-the-HAM-warm dummies: cheap bf16 matmuls on resident data
                for _ in range(n):
                    wm = ps.tile([1, 512], dt, tag="wm", bufs=1, name="wm")
                    nc.tensor.matmul(
                        wm[:], ones_bf[:], wscr[:], start=True, stop=True
                    )

            def rms(xt, name, out_dtype=dt, rowc=None):
                """x * rsqrt(mean(x^2) + eps) for x in [128, 16] column layout.

                rowc: [1, 128] broadcast row; its value multiplies the
                rsqrt scale (used to fold the fp8 weight pre-scale in).
                """
                sq = small.tile([128, 16], dt, tag=f"sq_{name}")
                ssum = small.tile([128, 1], dt, tag=f"ss_{name}")
                nc.scalar.activation(sq[:], xt[:], AF.Square, accum_out=ssum[:])
                tot = ps.tile([1, 1], dt, tag="b")
                nc.tensor.matmul(tot[:], ssum[:], ones_col[:], start=True, stop=True)
                std = small.tile([1, 1], dt, tag=f"std_{name}")
                nc.scalar.activation(
                    std[:], tot[:], AF.Sqrt, bias=eps_c[:], scale=1.0 / float(E)
                )
                inv = small.tile([1, 1], dt, tag=f"inv_{name}")
                nc.vector.reciprocal(inv[:], std[:])
                invb_ps = ps.tile([128, 1], dt, tag="b")
                nc.tensor.matmul(
                    invb_ps[:], rowc if rowc is not None else ones_row[:],
                    inv[:], start=True, stop=True,
                )
                xn = small.tile([128, 16], out_dtype, tag=f"xn_{name}")
                nc.vector.tensor_scalar_mul(xn[:], xt[:], invb_ps[:])
                return xn

            x0 = small.tile([128, 16], dt, tag="x0")
            nc.vector.tensor_add(x0[:], xw[:], xp[:])
            x1 = rms(x0, "n1")  # residual input (fp32)
            # second rms emits bf16 with the wqkv fp8 pre-scale folded in
            x2b = rms(x1, "n2", out_dtype=bf, rowc=ones_row_ws[:])

            # ---- qkv projection: [1, 768] row (q 0:256 | k 256:512 | v 512:768)
            wqkv_t = stile("qkv", 16 * 768, f8)
            nc.sync.dma_start(wqkv_t[:], wqkv_r[:])
            ps_q = ps.tile([1, 512], dt, tag="b", name="ps_q")
            ps_v = ps.tile([1, 256], dt, tag="b", name="ps_v")
            for k in range(16):
                wt = wqkv_t[:, k * 768 : (k + 1) * 768]
                nc.tensor.matmul(
                    ps_q[:], x2b[:, k : k + 1], wt[:, 0:512],
                    start=(k == 0), stop=(k == 15),
                )
                nc.tensor.matmul(
                    ps_v[:], x2b[:, k : k + 1], wt[:, 512:768],
                    start=(k == 0), stop=(k == 15),
                )
            qkv_row = small.tile([1, 768], dt, tag="qkv")
            nc.scalar.mul(qkv_row[:, 0:256], ps_q[:, 0:256], SCALE)
            nc.scalar.copy(qkv_row[:, 256:512], ps_q[:, 256:512])
            nc.scalar.copy(qkv_row[:, 512:768], ps_v[:])

            # ---- transpose q,k to column layout: qkT[:, 0:2]=q heads, 2:4=k heads
            st4 = stage.tile([4, 128], dt, tag="st4")
            nc.scalar.dma_start(st4[:], qkv_row[:, 0:512])
            qkT_ps = ps.tile([128, 4], dt, tag="b")
            nc.tensor.transpose(qkT_ps[:], st4[:], ident[0:4, 0:4])
            qkT = small.tile([128, 4], bf, tag="qkT")
            nc.vector.tensor_copy(qkT[:], qkT_ps[:])

            # ---- attention scores, directly transposed: each 128-wide K block
            # is the PE stationary operand, q the (N=1) moving operand, so
            # att lands as [t-in-block, block] columns with no SBUF reshape.
            # wTboth[p, h*64 + c] = exp(att_h[c*128 + p])  (bf16 PV columns)
            wTboth = small.tile([128, 128], bf, tag="wTboth")
            esp2 = small.tile([128, 2], dt, tag="esp2")  # per-partition exp sums
            for h in range(HPC):
                kt = stile("key", 8192, f8)
                nc.sync.dma_start(kt[:], keys_r[:, h * 8192 : (h + 1) * 8192])
                att_ps = ps.tile([128, 64], dt, tag="b", name=f"attps{h}")
                for b in range(64):
                    nc.tensor.matmul(
                        att_ps[:, b : b + 1],
                        kt[:, b * 128 : (b + 1) * 128],
                        qkT[:, h : h + 1],
                        start=True, stop=True,
                    )
                nc.scalar.activation(
                    wTboth[:, h * 64 : (h + 1) * 64], att_ps[:], AF.Exp,
                    accum_out=esp2[:, h : h + 1],
                )

            # current-token score per head: exp(q_h . k_h) (SCALE folded into q)
            e_last = small.tile([1, 2], dt, tag="elast")
            for h in range(HPC):
                pal = ps.tile([1, 1], dt, tag="b")
                nc.tensor.matmul(
                    pal[:], qkT[:, h : h + 1], qkT[:, 2 + h : 3 + h],
                    start=True, stop=True,
                )
                nc.scalar.activation(e_last[:, h : h + 1], pal[:], AF.Exp)

            # softmax denominators: cross-partition sum of esp2 + e_last
            dps = ps.tile([1, 2], dt, tag="b")
            nc.tensor.matmul(dps[:], ones_col[:], esp2[:], start=True, stop=True)
            dtmp = small.tile([1, 2], dt, tag="dtmp")
            nc.vector.tensor_add(dtmp[:], dps[:], e_last[:])
            dinv = small.tile([1, 2], dt, tag="dinv")
            nc.vector.reciprocal(dinv[:], dtmp[:])

            # ---- PV on the PE: lhsT = exp column [128, 1] (t-block on
            # partitions), rhs = V block [t, d]; 64 accumulating N=128
            # matmuls per head.
            pv_ps = [
                ps.tile([1, 128], dt, tag="b", name=f"pv_ps{h}") for h in range(HPC)
            ]
            for tt in range(2):
                vt = stile("val", 8192, f8)
                nc.sync.dma_start(vt[:], vals_r[:, tt * 8192 : (tt + 1) * 8192])
                for j in range(32):
                    c = tt * 32 + j
                    for h in range(HPC):
                        nc.tensor.matmul(
                            pv_ps[h][:],
                            wTboth[:, h * 64 + c : h * 64 + c + 1],
                            vt[:, j * 256 + h * 128 : j * 256 + (h + 1) * 128],
                            start=(c == 0), stop=(c == 63),
                        )

            # combine with current-token value, then normalize by the softmax sum
            xa_row = small.tile([1, 256], dt, tag="xa")
            for h in range(HPC):
                sl = slice(h * 128, (h + 1) * 128)
                nc.vector.tensor_scalar_mul(
                    xa_row[:, sl],
                    qkv_row[:, 512 + h * 128 : 512 + (h + 1) * 128],
                    e_last[:, h : h + 1],
                )
                nc.vector.tensor_add(xa_row[:, sl], xa_row[:, sl], pv_ps[h][:])
                nc.vector.tensor_scalar_mul(xa_row[:, sl], xa_row[:, sl], dinv[:, h : h + 1])

            # ---- transpose x_attn to column layout [128, 2], fold wo pre-scale
            st2 = stage.tile([2, 128], dt, tag="st2")
            nc.scalar.dma_start(st2[:], xa_row[:])
            xaT_ps = ps.tile([128, 2], dt, tag="b")
            nc.tensor.transpose(xaT_ps[:], st2[:], ident[0:2, 0:2])
            xaT = small.tile([128, 2], bf, tag="xaT")
            nc.scalar.mul(xaT[:], xaT_ps[:], 1.0 / WS)

            # ---- wo partial: [1, 2048] = 4 PE chains over 2 k-columns
            ar1_in = small.tile([1, 2048], bf, tag="arin", name="ar1_in")
            wot = stile("wo", 4096, f8)
            nc.sync.dma_start(wot[:], wo_r[:])
            wo_pe = [ps.tile([1, 512], dt, tag="b", name=f"wope{n}") for n in range(4)]
            for k in range(2):
                for n in range(4):
                    nc.tensor.matmul(
                        wo_pe[n][:], xaT[:, k : k + 1],
                        wot[:, k * 2048 + n * 512 : k * 2048 + (n + 1) * 512],
                        start=(k == 0), stop=(k == 1),
                    )
            for n in range(4):
                if n % 2 == 0:
                    nc.vector.tensor_copy(ar1_in[:, n * 512 : (n + 1) * 512], wo_pe[n][:])
                else:
                    nc.scalar.copy(ar1_in[:, n * 512 : (n + 1) * 512], wo_pe[n][:])

            def all_reduce(row_sb, idx, name):
                """Sum a [1, 2048] partial across cores; returns PSUM [128, 16].

                AllGather + on-core rank reduction: 16 K=8 matmuls against a
                ones vector sum the 8 gathered rows and transpose into the
                [128, 16] column layout.
                """
                in_d = dram.tile([1, 2048], bf, tag=f"{name}_in")
                out_d = dram.tile([N_CORES, 2048], bf, tag=f"{name}_out")
                nc.scalar.dma_start(in_d[:], row_sb[:])
                nc.gpsimd.collective_compute(
                    "AllGather",
                    mybir.AluOpType.bypass,
                    replica_groups=[list(range(N_CORES))],
                    ins=[in_d.opt()],
                    outs=[out_d.opt()],
                )
                ag_sb = stage.tile(
                    [N_CORES, 2048], bf, tag="ag", name=f"ag_{name}", bufs=1
                )
                nc.scalar.dma_start(ag_sb[:], out_d[:])
                x_ps = ps.tile([128, 16], dt, tag="b", name=f"xps_{name}")
                for c in range(16):
                    nc.tensor.matmul(
                        x_ps[:, c : c + 1],
                        ag_sb[:, c * 128 : (c + 1) * 128],
                        ones_bf[0:N_CORES, :],
                        start=True, stop=True,
                    )
                return x_ps

            x3_ps = all_reduce(ar1_in, 0, "ar1")
            x3 = small.tile([128, 16], dt, tag="x3")
            nc.vector.tensor_add(x3[:], x3_ps[:], x1[:])  # + residual

            # ---- MLP1: h = relu(w1 @ x4), 2 PE chains over 16 k-columns ----
            x4b = rms(x3, "n3", out_dtype=bf)
            ph = [ps.tile([1, 512], dt, tag="b", name=f"ph{n}") for n in range(2)]
            for a in range(2):
                w1t = stile("w1", 8192, bf)
                nc.sync.dma_start(w1t[:], w1_r[:, a * 8192 : (a + 1) * 8192])
                for b in range(8):
                    k = a * 8 + b
                    for n in range(2):
                        nc.tensor.matmul(
                            ph[n][:], x4b[:, k : k + 1],
                            w1t[:, b * 1024 + n * 512 : b * 1024 + (n + 1) * 512],
                            start=(k == 0), stop=(k == 15),
                        )
            # relu + transpose per 512-col half so MLP2's first k-group can
            # start while the second half is still in its SBUF-DMA hop
            h_row = small.tile([1, 1024], dt, tag="hrow")
            hT = []
            for n in range(2):
                nc.scalar.activation(h_row[:, n * 512 : (n + 1) * 512], ph[n][:], AF.Relu)
                st4h = stage.tile([4, 128], dt, tag="st4h", name=f"st4h{n}", bufs=2)
                nc.scalar.dma_start(st4h[:], h_row[:, n * 512 : (n + 1) * 512])
                hT_ps = ps.tile([128, 4], dt, tag="b", name=f"hTps{n}")
                nc.tensor.transpose(hT_ps[:], st4h[:], ident[0:4, 0:4])
                ht = small.tile([128, 4], bf, tag="hT", name=f"hT{n}", bufs=2)
                nc.vector.tensor_copy(ht[:], hT_ps[:])
                hT.append(ht)

            # ---- MLP2: [1, 2048] = 4 PE chains over 8 k-columns ----
            ar2_in = small.tile([1, 2048], bf, tag="arin", name="ar2_in")
            pm = [ps.tile([1, 512], dt, tag="b", name=f"pm{n}") for n in range(4)]
            for a in range(2):
                w2t = stile("w2", 8192, bf)
                nc.sync.dma_start(w2t[:], w2_r[:, a * 8192 : (a + 1) * 8192])
                for b in range(4):
                    k = a * 4 + b
                    for n in range(4):
                        nc.tensor.matmul(
                            pm[n][:], hT[k // 4][:, k % 4 : k % 4 + 1],
                            w2t[:, b * 2048 + n * 512 : b * 2048 + (n + 1) * 512],
                            start=(k == 0), stop=(k == 7),
                        )
            for n in range(4):
                if n % 2 == 0:
                    nc.vector.tensor_copy(ar2_in[:, n * 512 : (n + 1) * 512], pm[n][:])
                else:
                    nc.scalar.copy(ar2_in[:, n * 512 : (n + 1) * 512], pm[n][:])

            x5_ps = all_reduce(ar2_in, 1, "ar2")
            x5f = small.tile([128, 16], dt, tag="x5f")
            nc.vector.tensor_add(x5f[:], x5_ps[:], x3[:])  # + residual (x3)
            x5b = small.tile([128, 16], bf, tag="x5b")
            nc.scalar.copy(x5b[:], x5f[:])

            # ---- LM head over the vocab shard: 13 k-innermost column blocks,
            # each one contiguous DMA. PE blocks: one 16-matmul PSUM chain.
            # DVE blocks: 16 scalar_tensor_tensor accumulates into an SBUF acc
            # with the cross-partition ones-matmul reduce deferred so the PE
            # never stalls waiting on the DVE.
            def drain(pl, lo, w, cb):
                lr = small.tile([1, 512], dt, tag="lrow", name=f"lr{cb}", bufs=3)
                nc.vector.tensor_copy(lr[:, 0:w], pl[:])
                nc.scalar.dma_start(logits_out[:, lo : lo + w], lr[:, 0:w])

            pending = []  # deferred DVE-block reduces: (acc, lo, w, cb)
            pe_seen = 0
            off16 = 0
            for cb, (lo, w) in enumerate(LM_BLOCKS):
                lt = stile("lm", 16 * w, bf)
                nc.sync.dma_start(lt[:], lm_r[:, off16 : off16 + 16 * w])
                if cb in DVE_BLOCKS:
                    # bf16 accumulator: both DVE ports 16-bit
                    acc = accp.tile([128, w], bf, tag="acc", name=f"acc{cb}")
                    for k in range(16):
                        if k == 0:
                            nc.vector.tensor_scalar_mul(
                                acc[:], lt[:, 0:w], x5f[:, 0:1]
                            )
                        else:
                            nc.vector.scalar_tensor_tensor(
                                acc[:], lt[:, k * w : (k + 1) * w],
                                x5f[:, k : k + 1], acc[:],
                                op0=MUL, op1=ADD,
                            )
                    pending.append((acc, lo, w, cb))
                else:
                    pe_seen += 1
                    if pe_seen in (4, 8) and pending:
                        acc, plo, pw, pcb = pending.pop(0)
                        pr = ps.tile([1, pw], dt, tag="b", name=f"pr{pcb}")
                        nc.tensor.matmul(
                            pr[:], ones_bf[:], acc[:], start=True, stop=True
                        )
                        drain(pr, plo, pw, pcb)
                    pl = ps.tile([1, w], dt, tag="b", name=f"pl{cb}")
                    for k in range(16):
                        nc.tensor.matmul(
                            pl[:], x5b[:, k : k + 1], lt[:, k * w : (k + 1) * w],
                            start=(k == 0), stop=(k == 15),
                        )
                    drain(pl, lo, w, cb)
                off16 += 16 * w
            assert not pending, "deferred DVE reduces must drain before cb12"

            # warmup-collective readback + DCE-keeper. The pin copy below
            # writes the destination tile from x5f first, so the readback
            # DMA (WAW on that tile) cannot be scheduled into any engine
            # FIFO until after AR2 -- otherwise the scheduler hoists it and
            # the ACT queue blocks mid-kernel on the warmup AG.
            warm_back = stage.tile([1, 16], dt, tag="warmb", bufs=1)
            nc.vector.tensor_copy(warm_back[:], x5f[0:1, :])  # ordering pin
            nc.scalar.dma_start(warm_back[:], warm_out[0:1, :])
            klo, kw = LM_BLOCKS[-1]
            kr = small.tile([1, 16], dt, tag="keep")
            nc.vector.scalar_tensor_tensor(
                kr[:], warm_back[:], 0.0, warm_back[:], op0=MUL, op1=MUL
            )
            nc.gpsimd.dma_start(
                logits_out[:, klo + kw - 16 : klo + kw], kr[:],
                accum_op=ADD,
            )

    nc.finalize()
    return nc


def _col16(v):
    """[2048] vector -> [128, 16] column-major layout (e = c*128 + p at [p, c])."""
    return np.ascontiguousarray(v.reshape(16, 128).T)


def _part_major(mT, nblk, blk_rows, width):
    """[nblk*blk_rows, width] -> [blk_rows, nblk*width] partition-major."""
    return np.ascontiguousarray(
        mT.reshape(nblk, blk_rows, width).transpose(1, 0, 2).reshape(blk_rows, nblk * width)
    )


def _to_f8(a):
    import ml_dtypes

    return np.clip(a, -240.0, 240.0).astype(ml_dtypes.float8_e4m3)


def _to_bf(a):
    import ml_dtypes

    return a.astype(ml_dtypes.bfloat16)


def _lm_blocked(shard):
    """[VPC, E] fp32 -> [128, 16*VPC] bf16 with k-innermost 512-col blocks."""
    cols = []
    for lo, w in LM_BLOCKS:
        blk = shard[lo : lo + w].T  # [E, w]
        cols.append(blk.reshape(16, 128, w).transpose(1, 0, 2).reshape(128, 16 * w))
    return _to_bf(np.concatenate(cols, axis=1))


def _prep_in_maps(token_id, pos_id, keys, values, wte, wpe, wq, wk, wv, wo, w1, w2, lm_w):
    f32 = lambda a: np.asarray(a, dtype=np.float32)
    keys, values = f32(keys), f32(values)
    wq, wk, wv, wo, w1, w2, lm_w = map(f32, (wq, wk, wv, wo, w1, w2, lm_w))
    xe_wte = _col16(f32(wte[token_id]))
    xe_wpe = _col16(f32(wpe[pos_id]))
    lm_pad = np.zeros((N_CORES * VPC, E), np.float32)
    lm_pad[:VOCAB] = lm_w

    in_maps = []
    for i in range(N_CORES):
        hs = slice(i * EPC, (i + 1) * EPC)
        wqkv = np.concatenate([wq[hs], wk[hs], wv[hs]], axis=0)  # [768, E]
        in_maps.append(
            {
                "xe_wte": xe_wte,
                "xe_wpe": xe_wpe,
                "wqkv_r": _to_f8(
                    _part_major(np.ascontiguousarray(wqkv.T) * WS, 16, 128, 768)
                ),
                "keys_r": _to_f8(
                    _part_major(np.ascontiguousarray(keys[:, hs].T), 2, 128, 8192)
                ),
                "vals_r": _to_f8(_part_major(values[:, hs], 64, 128, EPC)),
                "wo_r": _to_f8(
                    _part_major(np.ascontiguousarray(wo[:, hs].T) * WS, 2, 128, E)
                ),
                "w1_r": _to_bf(
                    _part_major(
                        np.ascontiguousarray(w1[i * 1024 : (i + 1) * 1024].T),
                        16, 128, 1024,
                    )
                ),
                "w2_r": _to_bf(
                    _part_major(
                        np.ascontiguousarray(w2[:, i * 1024 : (i + 1) * 1024].T),
                        8, 128, E,
                    )
                ),
                "lm_r": _lm_blocked(lm_pad[i * VPC : (i + 1) * VPC]),
            }
        )
    return in_maps


def kernel(**inputs) -> np.ndarray:
    from concourse.bass_utils import run_bass_kernel_spmd

    token_id = int(inputs["token_id"])
    pos_id = int(inputs["pos_id"])
    in_maps = _prep_in_maps(
        token_id,
        pos_id,
        inputs["keys"],
        inputs["values"],
        inputs["wte"],
        inputs["wpe"],
        inputs["wq"],
        inputs["wk"],
        inputs["wv"],
        inputs["wo"],
        inputs["w1"],
        inputs["w2"],
        inputs["lm_w"],
    )
    if "nc" not in _CACHE:
        _CACHE["nc"] = _build_nc()
    nc = _CACHE["nc"]
    res = run_bass_kernel_spmd(
        nc,
        in_maps,
        core_ids=list(range(N_CORES)),
        trace=TRACE,
        trace_cores=[0] if TRACE else None,
    )
    _CACHE["last_result"] = res
    logits = np.concatenate([r["logits"][0] for r in res.results])[:VOCAB]
    return np.ascontiguousarray(logits.astype(np.float32))


# revision 46
# speedup vs baseline: 1.2493x; 1.0338x over previous
"""Tensor-parallel MiniGPT single-token decode step on 8 Trainium2 NeuronCores.

Sharding (per core i of 8):
  - attention: heads 2i, 2i+1 (head_dim 128 -> cols i*256:(i+1)*256 of E=2048);
    wq/wk/wv row-sharded, wo column-sharded, KV cache column-sharded by head.
  - MLP: w1 row-sharded (1024 rows/core), w2 column-sharded.
  - LM head: vocab-sharded (50257 padded to 8*6283=50264 rows).
  - Two 4KB bf16 AllGathers + on-core rank reduction combine the wo- and
    w2- partial sums; logits are gathered on the host.

Memory-bound regime: all streamed weights are narrow on the wire.
  - fp8 e4m3: wqkv, K cache, V cache, wo  (attention output is ~1% of the
    residual stream, so 3.6% quantization RMS there is ~4e-4 end to end).
    wqkv/wo are pre-scaled x1024 on the host so sigma=0.02 weights land in
    e4m3's normal range; the 1/1024 is folded into the bf16 activation.
  - bf16: w1, w2, lm_head (these feed logits directly; bf16 keeps ~1e-3).

Engine plan: every matvec contraction runs on the PE (bf16/fp8 moving
operand streams 128 elements/cycle); attention scores are computed
directly in transposed [t-in-block, block] form by loading each K block
as the PE stationary operand, which kills the row->column SBUF-DMA
transpose pipeline. PV accumulates per head with N=128 matmuls
(lhsT = one exp column per t-block). The lm_head shard is k-innermost 512-column
blocks (one contiguous 2MB DMA = one PSUM chain); two of the 13 blocks
run on the DVE (scalar_tensor_tensor accumulate + a deferred ones-matmul
partition reduce) so the post-AllReduce tail is not PE-serial. One shared
9-slot SBUF stream ring lets the lm stream run ahead ~10MB while the
AllGathers are in flight, keeping the DMA queues busy end to end.
"""

import numpy as np

N_CORES = 8
E = 2048
HPC = 2  # heads per core
EPC = HPC * 128  # 256
T = 8192
VOCAB = 50257
VPC = 6283  # padded vocab rows per core (8 * 6283 = 50264)
SCALE = float(1.0 / np.sqrt(128.0))
EPS = 1e-5
WS = 1024.0  # fp8 pre-scale for wqkv / wo

# lm_head chain blocks: (col offset within shard, width)
LM_BLOCKS = [(i * 512, 512) for i in range(12)] + [(6144, VPC - 6144)]
DVE_BLOCKS = (0, 2)  # lm blocks accumulated on the DVE instead of PE

_CACHE = {}
TRACE = False


def _build_nc():
    import concourse.bacc as bacc
    import concourse.mybir as mybir
    import concourse.tile as tile
    from concourse.masks import make_identity

    AF = mybir.ActivationFunctionType
    MUL = mybir.AluOpType.mult
    ADD = mybir.AluOpType.add
    dt = mybir.dt.float32
    bf = mybir.dt.bfloat16
    f8 = mybir.dt.float8e4

    nc = bacc.Bacc(
        "TRN2", target_bir_lowering=False, debug=False, num_devices=N_CORES
    )

    xe_wte = nc.declare_dram_parameter("xe_wte", [128, 16], dt, isOutput=False)
    xe_wpe = nc.declare_dram_parameter("xe_wpe", [128, 16], dt, isOutput=False)
    wqkv_r = nc.declare_dram_parameter("wqkv_r", [128, 16 * 768], f8, isOutput=False)
    keys_r = nc.declare_dram_parameter("keys_r", [128, 2 * 8192], f8, isOutput=False)
    vals_r = nc.declare_dram_parameter("vals_r", [128, 64 * 256], f8, isOutput=False)
    wo_r = nc.declare_dram_parameter("wo_r", [128, 2 * 2048], f8, isOutput=False)
    w1_r = nc.declare_dram_parameter("w1_r", [128, 16 * 1024], bf, isOutput=False)
    w2_r = nc.declare_dram_parameter("w2_r", [128, 8 * 2048], bf, isOutput=False)
    lm_r = nc.declare_dram_parameter("lm_r", [128, 16 * VPC], bf, isOutput=False)
    logits_out = nc.declare_dram_parameter("logits", [1, VPC], dt, isOutput=True)

    with tile.TileContext(nc) as tc:
        with (
            tc.tile_pool(name="const", bufs=1) as const,
            tc.tile_pool(name="small", bufs=1) as small,
            tc.tile_pool(name="stage", bufs=2) as stage,
            tc.tile_pool(name="ps", bufs=7, space="PSUM") as ps,
            tc.tile_pool(name="dram", bufs=1, space="DRAM") as dram,
            tc.tile_pool(name="stream", bufs=9) as stream,
            tc.tile_pool(name="acc", bufs=2) as accp,
        ):
            _snum = [0]

            def stile(label, width, dtype):
                # one shared ring of slots; slot size = max tile = 16KB/part
                _snum[0] += 1
                return stream.tile(
                    [128, width], dtype, tag="s", name=f"s{_snum[0]}_{label}"
                )

            # Warm up the collectives path first: a NEFF's first collective
            # pays a large ncfw init (~60-80us observed) that would otherwise
            # land on AR1's critical path. Its output readback + DCE-keeper
            # are emitted at the very END of the program so no engine FIFO
            # ever blocks on this collective's completion.
            warm_in = dram.tile([1, 16], dt, tag="warm_in")
            warm_out = dram.tile([N_CORES, 16], dt, tag="warm_out")
            warm_sb = stage.tile([1, 16], dt, tag="warm", bufs=1)
            nc.vector.memset(warm_sb[:], 0.0)
            nc.gpsimd.dma_start(warm_in[:], warm_sb[:])
            nc.gpsimd.collective_compute(
                "AllGather",
                mybir.AluOpType.bypass,
                replica_groups=[list(range(N_CORES))],
                ins=[warm_in.opt()],
                outs=[warm_out.opt()],
            )

            # ---- embedding row loads lead the weight-stream queue ----
            xw = stage.tile([128, 16], dt, tag="xw")
            nc.sync.dma_start(xw[:], xe_wte[:])
            xp = stage.tile([128, 16], dt, tag="xp")
            nc.sync.dma_start(xp[:], xe_wpe[:])

            ones_col = const.tile([128, 1], dt)
            nc.vector.memset(ones_col[:], 1.0)
            ones_row = const.tile([1, 128], dt)
            nc.vector.memset(ones_row[:], 1.0)
            ones_row_ws = const.tile([1, 128], dt)
            nc.vector.memset(ones_row_ws[:], 1.0 / WS)
            ident = const.tile([16, 16], dt)
            make_identity(nc, ident[:])
            eps_c = const.tile([1, 1], dt)
            nc.vector.memset(eps_c[:], EPS)
            ones_bf = const.tile([128, 1], bf)
            nc.vector.memset(ones_bf[:], 1.0)

            wscr = const.tile([128, 512], bf)
            nc.vector.memset(wscr[:], 0.25)

            def warm(n):
                # keep-the-HAM-warm dummies: cheap bf16 matmuls on resident data
                for _ in range(n):
                    wm = ps.tile([1, 512], dt, tag="wm", bufs=1, name="wm")
                    nc.tensor.matmul(
                        wm[:], ones_bf[:], wscr[:], start=True, stop=True
                    )

            def rms(xt, name, out_dtype=dt, rowc=None):
                """x * rsqrt(mean(x^2) + eps) for x in [128, 16] column layout.

                rowc: [1, 128] broadcast row; its value multiplies the
                rsqrt scale (used to fold the fp8 weight pre-scale in).
                """
                sq = small.tile([128, 16], dt, tag=f"sq_{name}")
                ssum = small.tile([128, 1], dt, tag=f"ss_{name}")
                nc.scalar.activation(sq[:], xt[:], AF.Square, accum_out=ssum[:])
                tot = ps.tile([1, 1], dt, tag="b")
                nc.tensor.matmul(tot[:], ssum[:], ones_col[:], start=True, stop=True)
                std = small.tile([1, 1], dt, tag=f"std_{name}")
                nc.scalar.activation(
                    std[:], tot[:], AF.Sqrt, bias=eps_c[:], scale=1.0 / float(E)
                )
                inv = small.tile([1, 1], dt, tag=f"inv_{name}")
                nc.vector.reciprocal(inv[:], std[:])
                invb_ps = ps.tile([128, 1], dt, tag="b")
                nc.tensor.matmul(
                    invb_ps[:], rowc if rowc is not None else ones_row[:],
                    inv[:], start=True, stop=True,
                )
                xn = small.tile([128, 16], out_dtype, tag=f"xn_{name}")
                nc.vector.tensor_scalar_mul(xn[:], xt[:], invb_ps[:])
                return xn

            x0 = small.tile([128, 16], dt, tag="x0")
            nc.vector.tensor_add(x0[:], xw[:], xp[:])
            x1 = rms(x0, "n1")  # residual input (fp32)
            # second rms emits bf16 with the wqkv fp8 pre-scale folded in
            x2b = rms(x1, "n2", out_dtype=bf, rowc=ones_row_ws[:])

            # ---- qkv projection: [1, 768] row (q 0:256 | k 256:512 | v 512:768)
            wqkv_t = stile("qkv", 16 * 768, f8)
            nc.sync.dma_start(wqkv_t[:], wqkv_r[:])
            ps_q = ps.tile([1, 512], dt, tag="b", name="ps_q")
            ps_v = ps.tile([1, 256], dt, tag="b", name="ps_v")
            for k in range(16):
                wt = wqkv_t[:, k * 768 : (k + 1) * 768]
                nc.tensor.matmul(
                    ps_q[:], x2b[:, k : k + 1], wt[:, 0:512],
                    start=(k == 0), stop=(k == 15),
                )
                nc.tensor.matmul(
                    ps_v[:], x2b[:, k : k + 1], wt[:, 512:768],
                    start=(k == 0), stop=(k == 15),
                )
            qkv_row = small.tile([1, 768], dt, tag="qkv")
            nc.scalar.mul(qkv_row[:, 0:256], ps_q[:, 0:256], SCALE)
            nc.scalar.copy(qkv_row[:, 256:512], ps_q[:, 256:512])
            nc.scalar.copy(qkv_row[:, 512:768], ps_v[:])

            # ---- transpose q,k to column layout: qkT[:, 0:2]=q heads, 2:4=k heads
            st4 = stage.tile([4, 128], dt, tag="st4")
            nc.scalar.dma_start(st4[:], qkv_row[:, 0:512])
            qkT_ps = ps.tile([128, 4], dt, tag="b")
            nc.tensor.transpose(qkT_ps[:], st4[:], ident[0:4, 0:4])
            qkT = small.tile([128, 4], bf, tag="qkT")
            nc.vector.tensor_copy(qkT[:], qkT_ps[:])

            # ---- attention scores, directly transposed: each 128-wide K block
            # is the PE stationary operand, q the (N=1) moving operand, so
            # att lands as [t-in-block, block] columns with no SBUF reshape.
            # wTboth[p, h*64 + c] = exp(att_h[c*128 + p])  (bf16 PV columns)
            wTboth = small.tile([128, 128], bf, tag="wTboth")
            esp2 = small.tile([128, 2], dt, tag="esp2")  # per-partition exp sums
            for h in range(HPC):
                kt = stile("key", 8192, f8)
                nc.sync.dma_start(kt[:], keys_r[:, h * 8192 : (h + 1) * 8192])
                att_ps = ps.tile([128, 64], dt, tag="b", name=f"attps{h}")
                for b in range(64):
                    nc.tensor.matmul(
                        att_ps[:, b : b + 1],
                        kt[:, b * 128 : (b + 1) * 128],
                        qkT[:, h : h + 1],
                        start=True, stop=True,
                    )
                nc.scalar.activation(
                    wTboth[:, h * 64 : (h + 1) * 64], att_ps[:], AF.Exp,
                    accum_out=esp2[:, h : h + 1],
                )

            # current-token score per head: exp(q_h . k_h) (SCALE folded into q)
            e_last = small.tile([1, 2], dt, tag="elast")
            for h in range(HPC):
                pal = ps.tile([1, 1], dt, tag="b")
                nc.tensor.matmul(
                    pal[:], qkT[:, h : h + 1], qkT[:, 2 + h : 3 + h],
                    start=True, stop=True,
                )
                nc.scalar.activation(e_last[:, h : h + 1], pal[:], AF.Exp)

            # softmax denominators: cross-partition sum of esp2 + e_last
            dps = ps.tile([1, 2], dt, tag="b")
            nc.tensor.matmul(dps[:], ones_col[:], esp2[:], start=True, stop=True)
            dtmp = small.tile([1, 2], dt, tag="dtmp")
            nc.vector.tensor_add(dtmp[:], dps[:], e_last[:])
            dinv = small.tile([1, 2], dt, tag="dinv")
            nc.vector.reciprocal(dinv[:], dtmp[:])

            # ---- PV on the PE: lhsT = exp column [128, 1] (t-block on
            # partitions), rhs = V block [t, d]; 64 accumulating N=128
            # matmuls per head.
            pv_ps = [
                ps.tile([1, 128], dt, tag="b", name=f"pv_ps{h}") for h in range(HPC)
            ]
            for tt in range(2):
                vt = stile("val", 8192, f8)
                nc.sync.dma_start(vt[:], vals_r[:, tt * 8192 : (tt + 1) * 8192])
                for j in range(32):
                    c = tt * 32 + j
                    for h in range(HPC):
                        nc.tensor.matmul(
                            pv_ps[h][:],
                            wTboth[:, h * 64 + c : h * 64 + c + 1],
                            vt[:, j * 256 + h * 128 : j * 256 + (h + 1) * 128],
                            start=(c == 0), stop=(c == 63),
                        )

            # combine with current-token value, then normalize by the softmax sum
            xa_row = small.tile([1, 256], dt, tag="xa")
            for h in range(HPC):
                sl = slice(h * 128, (h + 1) * 128)
                nc.vector.tensor_scalar_mul(
                    xa_row[:, sl],
                    qkv_row[:, 512 + h * 128 : 512 + (h + 1) * 128],
                    e_last[:, h : h + 1],
                )
                nc.vector.tensor_add(xa_row[:, sl], xa_row[:, sl], pv_ps[h][:])
                nc.vector.tensor_scalar_mul(xa_row[:, sl], xa_row[:, sl], dinv[:, h : h + 1])

            # ---- transpose x_attn to column layout [128, 2], fold wo pre-scale
            st2 = stage.tile([2, 128], dt, tag="st2")
            nc.scalar.dma_start(st2[:], xa_row[:])
            xaT_ps = ps.tile([128, 2], dt, tag="b")
            nc.tensor.transpose(xaT_ps[:], st2[:], ident[0:2, 0:2])
            xaT = small.tile([128, 2], bf, tag="xaT")
            nc.scalar.mul(xaT[:], xaT_ps[:], 1.0 / WS)

            # ---- wo partial: [1, 2048] = 4 PE chains over 2 k-columns
            ar1_in = small.tile([1, 2048], bf, tag="arin", name="ar1_in")
            wot = stile("wo", 4096, f8)
            nc.sync.dma_start(wot[:], wo_r[:])
            wo_pe = [ps.tile([1, 512], dt, tag="b", name=f"wope{n}") for n in range(4)]
            for k in range(2):
                for n in range(4):
                    nc.tensor.matmul(
                        wo_pe[n][:], xaT[:, k : k + 1],
                        wot[:, k * 2048 + n * 512 : k * 2048 + (n + 1) * 512],
                        start=(k == 0), stop=(k == 1),
                    )
            for n in range(4):
                if n % 2 == 0:
                    nc.vector.tensor_copy(ar1_in[:, n * 512 : (n + 1) * 512], wo_pe[n][:])
                else:
                    nc.scalar.copy(ar1_in[:, n * 512 : (n + 1) * 512], wo_pe[n][:])

            def all_reduce(row_sb, idx, name):
                """Sum a [1, 2048] partial across cores; returns PSUM [128, 16].

                AllGather + on-core rank reduction: 16 K=8 matmuls against a
                ones vector sum the 8 gathered rows and transpose into the
                [128, 16] column layout.
                """
                in_d = dram.tile([1, 2048], bf, tag=f"{name}_in")
                out_d = dram.tile([N_CORES, 2048], bf, tag=f"{name}_out")
                nc.scalar.dma_start(in_d[:], row_sb[:])
                nc.gpsimd.collective_compute(
                    "AllGather",
                    mybir.AluOpType.bypass,
                    replica_groups=[list(range(N_CORES))],
                    ins=[in_d.opt()],
                    outs=[out_d.opt()],
                )
                ag_sb = stage.tile(
                    [N_CORES, 2048], bf, tag="ag", name=f"ag_{name}", bufs=1
                )
                nc.scalar.dma_start(ag_sb[:], out_d[:])
                x_ps = ps.tile([128, 16], dt, tag="b", name=f"xps_{name}")
                for c in range(16):
                    nc.tensor.matmul(
                        x_ps[:, c : c + 1],
                        ag_sb[:, c * 128 : (c + 1) * 128],
                        ones_bf[0:N_CORES, :],
                        start=True, stop=True,
                    )
                return x_ps

            x3_ps = all_reduce(ar1_in, 0, "ar1")
            x3 = small.tile([128, 16], dt, tag="x3")
            nc.vector.tensor_add(x3[:], x3_ps[:], x1[:])  # + residual

            # ---- MLP1: h = relu(w1 @ rms(x3)). ReLU commutes with the
            # positive rms scale, so the matmul chains run on the raw x3
            # (bf16 cast only) while the rsqrt scalar is computed
            # concurrently; the scale folds into the ReLU drain below.
            x3b = small.tile([128, 16], bf, tag="x3b")
            nc.vector.tensor_copy(x3b[:], x3[:])
            sq3 = small.tile([128, 16], dt, tag="sq_n3")
            ssum3 = small.tile([128, 1], dt, tag="ss_n3")
            nc.scalar.activation(sq3[:], x3[:], AF.Square, accum_out=ssum3[:])
            tot3 = ps.tile([1, 1], dt, tag="b")
            nc.tensor.matmul(tot3[:], ssum3[:], ones_col[:], start=True, stop=True)
            std3 = small.tile([1, 1], dt, tag="std_n3")
            nc.scalar.activation(
                std3[:], tot3[:], AF.Sqrt, bias=eps_c[:], scale=1.0 / float(E)
            )
            inv3 = small.tile([1, 1], dt, tag="inv_n3")
            nc.vector.reciprocal(inv3[:], std3[:])
            ph = [ps.tile([1, 512], dt, tag="b", name=f"ph{n}") for n in range(2)]
            for a in range(2):
                w1t = stile("w1", 8192, bf)
                nc.sync.dma_start(w1t[:], w1_r[:, a * 8192 : (a + 1) * 8192])
                for b in range(8):
                    k = a * 8 + b
                    for n in range(2):
                        nc.tensor.matmul(
                            ph[n][:], x3b[:, k : k + 1],
                            w1t[:, b * 1024 + n * 512 : b * 1024 + (n + 1) * 512],
                            start=(k == 0), stop=(k == 15),
                        )
            # relu + transpose per 512-col half so MLP2's first k-group can
            # start while the second half is still in its SBUF-DMA hop;
            # scale=inv3 applies the rms normalization (relu(u*s) = s*relu(u))
            h_row = small.tile([1, 1024], dt, tag="hrow")
            hT = []
            for n in range(2):
                nc.scalar.activation(
                    h_row[:, n * 512 : (n + 1) * 512], ph[n][:], AF.Relu,
                    scale=inv3[:],
                )
                st4h = stage.tile([4, 128], dt, tag="st4h", name=f"st4h{n}", bufs=2)
                nc.scalar.dma_start(st4h[:], h_row[:, n * 512 : (n + 1) * 512])
                hT_ps = ps.tile([128, 4], dt, tag="b", name=f"hTps{n}")
                nc.tensor.transpose(hT_ps[:], st4h[:], ident[0:4, 0:4])
                ht = small.tile([128, 4], bf, tag="hT", name=f"hT{n}", bufs=2)
                nc.vector.tensor_copy(ht[:], hT_ps[:])
                hT.append(ht)

            # ---- MLP2: [1, 2048] = 4 PE chains over 8 k-columns ----
            ar2_in = small.tile([1, 2048], bf, tag="arin", name="ar2_in")
            pm = [ps.tile([1, 512], dt, tag="b", name=f"pm{n}") for n in range(4)]
            for a in range(2):
                w2t = stile("w2", 8192, bf)
                nc.sync.dma_start(w2t[:], w2_r[:, a * 8192 : (a + 1) * 8192])
                for b in range(4):
                    k = a * 4 + b
                    for n in range(4):
                        nc.tensor.matmul(
                            pm[n][:], hT[k // 4][:, k % 4 : k % 4 + 1],
                            w2t[:, b * 2048 + n * 512 : b * 2048 + (n + 1) * 512],
                            start=(k == 0), stop=(k == 7),
                        )
            for n in range(4):
                if n % 2 == 0:
                    nc.vector.tensor_copy(ar2_in[:, n * 512 : (n + 1) * 512], pm[n][:])
                else:
                    nc.scalar.copy(ar2_in[:, n * 512 : (n + 1) * 512], pm[n][:])

            x5_ps = all_reduce(ar2_in, 1, "ar2")
            x5f = small.tile([128, 16], dt, tag="x5f")
            nc.vector.tensor_add(x5f[:], x5_ps[:], x3[:])  # + residual (x3)
            x5b = small.tile([128, 16], bf, tag="x5b")
            nc.scalar.copy(x5b[:], x5f[:])

            # ---- LM head over the vocab shard: 13 k-innermost column blocks,
            # each one contiguous DMA. PE blocks: one 16-matmul PSUM chain.
            # DVE blocks: 16 scalar_tensor_tensor accumulates into an SBUF acc
            # with the cross-partition ones-matmul reduce deferred so the PE
            # never stalls waiting on the DVE.
            def drain(pl, lo, w, cb):
                lr = small.tile([1, 512], dt, tag="lrow", name=f"lr{cb}", bufs=3)
                nc.vector.tensor_copy(lr[:, 0:w], pl[:])
                nc.scalar.dma_start(logits_out[:, lo : lo + w], lr[:, 0:w])

            pending = []  # deferred DVE-block reduces: (acc, lo, w, cb)
            pe_seen = 0
            off16 = 0
            for cb, (lo, w) in enumerate(LM_BLOCKS):
                lt = stile("lm", 16 * w, bf)
                nc.sync.dma_start(lt[:], lm_r[:, off16 : off16 + 16 * w])
                if cb in DVE_BLOCKS:
                    # bf16 accumulator: both DVE ports 16-bit
                    acc = accp.tile([128, w], bf, tag="acc", name=f"acc{cb}")
                    for k in range(16):
                        if k == 0:
                            nc.vector.tensor_scalar_mul(
                                acc[:], lt[:, 0:w], x5f[:, 0:1]
                            )
                        else:
                            nc.vector.scalar_tensor_tensor(
                                acc[:], lt[:, k * w : (k + 1) * w],
                                x5f[:, k : k + 1], acc[:],
                                op0=MUL, op1=ADD,
                            )
                    pending.append((acc, lo, w, cb))
                else:
                    pe_seen += 1
                    if pe_seen in (4, 8) and pending:
                        acc, plo, pw, pcb = pending.pop(0)
                        pr = ps.tile([1, pw], dt, tag="b", name=f"pr{pcb}")
                        nc.tensor.matmul(
                            pr[:], ones_bf[:], acc[:], start=True, stop=True
                        )
                        drain(pr, plo, pw, pcb)
                    pl = ps.tile([1, w], dt, tag="b", name=f"pl{cb}")
                    for k in range(16):
                        nc.tensor.matmul(
                            pl[:], x5b[:, k : k + 1], lt[:, k * w : (k + 1) * w],
                            start=(k == 0), stop=(k == 15),
                        )
                    drain(pl, lo, w, cb)
                off16 += 16 * w
            assert not pending, "deferred DVE reduces must drain before cb12"

            # warmup-collective readback + DCE-keeper. The pin copy below
            # writes the destination tile from x5f first, so the readback
            # DMA (WAW on that tile) cannot be scheduled into any engine
            # FIFO until after AR2 -- otherwise the scheduler hoists it and
            # the ACT queue blocks mid-kernel on the warmup AG.
            warm_back = stage.tile([1, 16], dt, tag="warmb", bufs=1)
            nc.vector.tensor_copy(warm_back[:], x5f[0:1, :])  # ordering pin
            nc.scalar.dma_start(warm_back[:], warm_out[0:1, :])
            klo, kw = LM_BLOCKS[-1]
            kr = small.tile([1, 16], dt, tag="keep")
            nc.vector.scalar_tensor_tensor(
                kr[:], warm_back[:], 0.0, warm_back[:], op0=MUL, op1=MUL
            )
            nc.gpsimd.dma_start(
                logits_out[:, klo + kw - 16 : klo + kw], kr[:],
                accum_op=ADD,
            )

    nc.finalize()
    return nc


def _col16(v):
    """[2048] vector -> [128, 16] column-major layout (e = c*128 + p at [p, c])."""
    return np.ascontiguousarray(v.reshape(16, 128).T)


def _part_major(mT, nblk, blk_rows, width):
    """[nblk*blk_rows, width] -> [blk_rows, nblk*width] partition-major."""
    return np.ascontiguousarray(
        mT.reshape(nblk, blk_rows, width).transpose(1, 0, 2).reshape(blk_rows, nblk * width)
    )


def _to_f8(a):
    import ml_dtypes

    return np.clip(a, -240.0, 240.0).astype(ml_dtypes.float8_e4m3)


def _to_bf(a):
    import ml_dtypes

    return a.astype(ml_dtypes.bfloat16)


def _lm_blocked(shard):
    """[VPC, E] fp32 -> [128, 16*VPC] bf16 with k-innermost 512-col blocks."""
    cols = []
    for lo, w in LM_BLOCKS:
        blk = shard[lo : lo + w].T  # [E, w]
        cols.append(blk.reshape(16, 128, w).transpose(1, 0, 2).reshape(128, 16 * w))
    return _to_bf(np.concatenate(cols, axis=1))


def _prep_in_maps(token_id, pos_id, keys, values, wte, wpe, wq, wk, wv, wo, w1, w2, lm_w):
    f32 = lambda a: np.asarray(a, dtype=np.float32)
    keys, values = f32(keys), f32(values)
    wq, wk, wv, wo, w1, w2, lm_w = map(f32, (wq, wk, wv, wo, w1, w2, lm_w))
    xe_wte = _col16(f32(wte[token_id]))
    xe_wpe = _col16(f32(wpe[pos_id]))
    lm_pad = np.zeros((N_CORES * VPC, E), np.float32)
    lm_pad[:VOCAB] = lm_w

    in_maps = []
    for i in range(N_CORES):
        hs = slice(i * EPC, (i + 1) * EPC)
        wqkv = np.concatenate([wq[hs], wk[hs], wv[hs]], axis=0)  # [768, E]
        in_maps.append(
            {
                "xe_wte": xe_wte,
                "xe_wpe": xe_wpe,
                "wqkv_r": _to_f8(
                    _part_major(np.ascontiguousarray(wqkv.T) * WS, 16, 128, 768)
                ),
                "keys_r": _to_f8(
                    _part_major(np.ascontiguousarray(keys[:, hs].T), 2, 128, 8192)
                ),
                "vals_r": _to_f8(_part_major(values[:, hs], 64, 128, EPC)),
                "wo_r": _to_f8(
                    _part_major(np.ascontiguousarray(wo[:, hs].T) * WS, 2, 128, E)
                ),
                "w1_r": _to_bf(
                    _part_major(
                        np.ascontiguousarray(w1[i * 1024 : (i + 1) * 1024].T),
                        16, 128, 1024,
                    )
                ),
                "w2_r": _to_bf(
                    _part_major(
                        np.ascontiguousarray(w2[:, i * 1024 : (i + 1) * 1024].T),
                        8, 128, E,
                    )
                ),
                "lm_r": _lm_blocked(lm_pad[i * VPC : (i + 1) * VPC]),
            }
        )
    return in_maps


def kernel(**inputs) -> np.ndarray:
    from concourse.bass_utils import run_bass_kernel_spmd

    token_id = int(inputs["token_id"])
    pos_id = int(inputs["pos_id"])
    in_maps = _prep_in_maps(
        token_id,
        pos_id,
        inputs["keys"],
        inputs["values"],
        inputs["wte"],
        inputs["wpe"],
        inputs["wq"],
        inputs["wk"],
        inputs["wv"],
        inputs["wo"],
        inputs["w1"],
        inputs["w2"],
        inputs["lm_w"],
    )
    if "nc" not in _CACHE:
        _CACHE["nc"] = _build_nc()
    nc = _CACHE["nc"]
    res = run_bass_kernel_spmd(
        nc,
        in_maps,
        core_ids=list(range(N_CORES)),
        trace=TRACE,
        trace_cores=[0] if TRACE else None,
    )
    _CACHE["last_result"] = res
    logits = np.concatenate([r["logits"][0] for r in res.results])[:VOCAB]
    return np.ascontiguousarray(logits.astype(np.float32))
